# revision 13
# baseline (speedup 1.0000x reference)
import os
import numpy as np

# ---- problem constants (hardcoded; kernel.py must be self-contained) ----
IMG, WS, SHIFT = 32, 8, 4
C, HEADS, DEPTH = 512, 16, 24
E_DIM, N_E, B = 256, 8192, 8
L = IMG * IMG            # 1024
NW = WS * WS             # 64 tokens per window
HD = C // HEADS          # 32
NWIN = (IMG // WS) ** 2  # 16
FH = 4 * C               # 2048
P = 128
VBLK = HD + 2            # 34 (32 vals + softmax-denominator col + pad)
VW = HEADS * VBLK        # 544
NT = L // P              # 8 token tiles
KC = C // P              # 4 k-tiles over C
KE = E_DIM // P          # 2 k-tiles over E_DIM
SCALE = HD ** -0.5
G = IMG // WS            # 4 windows per side

_DEPTH = int(os.environ.get("BT_DEPTH", DEPTH))
_NCORES = int(os.environ.get("BT_NCORES", 8))
_TP = os.environ.get("BT_TP", "dma")  # 'dma' or 'pe' transposes
_STAGE = int(os.environ.get("BT_STAGE", "99"))
_ATT = int(os.environ.get("BT_ATT", "3"))


# ---- host-side helpers (mirror reference.py) ----
def _rel_index():
    coords = np.stack(np.meshgrid(np.arange(WS), np.arange(WS), indexing='ij'))
    cf = coords.reshape(2, -1)
    rel = (cf[:, :, None] - cf[:, None, :]).transpose(1, 2, 0)
    rel[:, :, 0] += WS - 1
    rel[:, :, 1] += WS - 1
    rel[:, :, 0] *= 2 * WS - 1
    return rel.sum(-1)  # [NW, NW] int


def _shift_mask():
    img = np.zeros((IMG, IMG), np.float32)
    cnt = 0
    sl = (slice(0, -WS), slice(-WS, -SHIFT), slice(-SHIFT, None))
    for hs in sl:
        for ws_ in sl:
            img[hs, ws_] = cnt
            cnt += 1
    win = img.reshape(IMG // WS, WS, IMG // WS, WS).transpose(0, 2, 1, 3).reshape(-1, NW)
    diff = win[:, None, :] - win[:, :, None]
    return np.where(diff != 0, -100.0, 0.0).astype(np.float32)  # [NWIN, NW, NW]


def _win_perm():
    t = np.arange(L).reshape(IMG, IMG)
    wm = t.reshape(IMG // WS, WS, IMG // WS, WS).transpose(0, 2, 1, 3).reshape(-1)
    inv = np.empty(L, np.int64)
    inv[wm] = np.arange(L)
    return wm, inv  # wm: dev->raster, inv: raster->dev


_WM, _WM_INV = _win_perm()
_REL = _rel_index()
_MASK = _shift_mask()

# representative window index per pattern type for the shifted-layer mask:
#   0: interior (no mask), 1: right-column, 2: bottom-row, 3: corner
_PTYPE_WIN = [0, 3, 12, 15]
# window-pair (wp2) -> pair-type index into EE tiles (shift layers)
#   wp2 = 2a + c ; pair windows (4a+2c, 4a+2c+1)
_WP2_PT = [0 if (wp2 // 2) < 3 else 2 for wp2 in range(NT)]
for _wp2 in range(NT):
    if _wp2 % 2 == 1:
        _WP2_PT[_wp2] += 1
# per pair-type, the window type of (w01=0, w01=1)
_PT_WTYPES = [(0, 0), (0, 1), (2, 2), (2, 3)]
_WTYPE_MASK = {0: np.zeros((NW, NW), np.float32),
               1: None, 2: None, 3: None}


def _wtype_masks():
    m = {0: np.zeros((NW, NW), np.float32)}
    m[1] = _MASK[3]
    m[2] = _MASK[12]
    m[3] = _MASK[15]
    return m


def _prepare(inputs):
    import ml_dtypes
    bf16 = ml_dtypes.bfloat16
    f32 = lambda a: np.ascontiguousarray(a, dtype=np.float32)
    b16 = lambda a: np.ascontiguousarray(np.asarray(a, np.float32).astype(bf16))
    x = np.asarray(inputs['x'], np.float32)
    dec_w = np.asarray(inputs['dec_w'], np.float32)
    dec_b = np.asarray(inputs['dec_b'], np.float32)
    pos = np.asarray(inputs['pos_embed'], np.float32)[0]
    n1w = np.asarray(inputs['n1w'], np.float32)
    n1b = np.asarray(inputs['n1b'], np.float32)
    qkv_w = np.asarray(inputs['qkv_w'], np.float32)
    qkv_b = np.asarray(inputs['qkv_b'], np.float32)
    proj_w = np.asarray(inputs['proj_w'], np.float32)
    proj_b = np.asarray(inputs['proj_b'], np.float32)
    rel_bias = np.asarray(inputs['rel_bias'], np.float32)
    n2w = np.asarray(inputs['n2w'], np.float32)
    n2b = np.asarray(inputs['n2b'], np.float32)
    fc1_w = np.asarray(inputs['fc1_w'], np.float32)
    fc1_b = np.asarray(inputs['fc1_b'], np.float32)
    fc2_w = np.asarray(inputs['fc2_w'], np.float32)
    fc2_b = np.asarray(inputs['fc2_b'], np.float32)
    normf_w = np.asarray(inputs['normf_w'], np.float32)
    normf_b = np.asarray(inputs['normf_b'], np.float32)
    pred_w = np.asarray(inputs['pred_w'], np.float32)
    pred_b = np.asarray(inputs['pred_b'], np.float32)

    D = _DEPTH
    sh = {}
    sh['decw'] = b16(dec_w.T)                       # [E, C]
    sh['posb'] = f32((pos + dec_b[None, :])[_WM])   # [L, C]

    wqk = np.empty((D, C, 2 * C), bf16)
    bqk = np.zeros((D, P, 8), np.float32)
    wvp = np.zeros((D, C, VW), bf16)
    vb = np.zeros((D, P, VW), np.float32)
    ee = np.empty((D, 4, 4, P, 256), bf16)
    wp_ = np.empty((D, C, C), bf16)
    pb = np.empty((D, P, C), np.float32)
    f1 = np.empty((D, C, FH), bf16)
    f1b = np.empty((D, P, FH // P), np.float32)
    f2 = np.empty((D, FH, C), bf16)
    f2b = np.empty((D, P, C), np.float32)

    wtm = _wtype_masks()

    for i in range(D):
        Wm = qkv_w[i] * n1w[i][None, :]
        bm = qkv_w[i] @ n1b[i] + qkv_b[i]
        Wm = Wm.copy()
        bm = bm.copy()
        Wm[:C] *= SCALE
        bm[:C] *= SCALE
        wqk[i] = Wm[:2 * C].T.astype(bf16)
        bqk[i] = np.asarray(bm[:2 * C].reshape(8, P).T, np.float32)
        for h in range(HEADS):
            wvp[i][:, h * VBLK:h * VBLK + HD] = Wm[2 * C + h * HD:2 * C + (h + 1) * HD].T.astype(bf16)
            vb[i][:, h * VBLK:h * VBLK + HD] = bm[2 * C + h * HD:2 * C + (h + 1) * HD][None, :]
            vb[i][:, h * VBLK + HD] = 1.0
        # multiplicative attention bias: EE[pt, hs][p = w01*64 + tk, s*64 + tq]
        #   = exp(B_h[tq, tk] + mask[wtype][tq, tq->tk])  (transposed into [tk, tq])
        bias = rel_bias[i][_REL]                  # [tq, tk, HEADS]
        shift = (i % 2) == 1
        for pt in range(4):
            wt0, wt1 = _PT_WTYPES[pt]
            for rg in range(4):
                for hi in range(4):
                    h = hi * 4 + rg
                    a = bias[:, :, h].T           # [tk, tq]
                    for w01, wt in ((0, wt0), (1, wt1)):
                        m = wtm[wt].T if shift else wtm[0]
                        e = np.exp(a + m)         # [tk, tq]
                        ee[i, pt, rg, w01 * NW:(w01 + 1) * NW, hi * NW:(hi + 1) * NW] = e.astype(bf16)
        wp_[i] = proj_w[i].T.astype(bf16)
        pb[i] = np.broadcast_to(proj_b[i][None, :], (P, C))
        f1[i] = (fc1_w[i] * n2w[i][None, :]).T.astype(bf16)
        f1b[i] = np.asarray((fc1_w[i] @ n2b[i] + fc1_b[i]).reshape(FH // P, P).T, np.float32)
        f2[i] = fc2_w[i].T.astype(bf16)
        f2b[i] = np.broadcast_to(fc2_b[i][None, :], (P, C))

    sh['wqk'] = wqk
    sh['bqk'] = bqk
    sh['wvp'] = wvp
    sh['vb'] = vb
    sh['ee'] = np.ascontiguousarray(ee)
    sh['wp'] = wp_
    sh['pb'] = pb
    sh['f1'] = f1
    sh['f1b'] = f1b
    sh['f2'] = f2
    sh['f2b'] = f2b
    sh['nfw'] = f32(normf_w.reshape(KC, P).T)     # [P, KC]
    sh['nfb'] = f32(normf_b.reshape(KC, P).T)
    sh['pw'] = b16(pred_w.T)                      # [C, N_E]
    xts = [np.ascontiguousarray(x[c][_WM].T.astype(bf16)) for c in range(B)]
    flags = {
        'bqk0': bool(np.all(bqk == 0.0)),
        'pb0': bool(np.all(pb == 0.0)),
        'f2b0': bool(np.all(f2b == 0.0)),
    }
    return sh, xts, np.asarray(pred_b, np.float32), flags


# ---- device program ----
_BUILD_CACHE = {}


def _build(flags):
    key = (_DEPTH, _TP, _STAGE, _ATT, flags['bqk0'], flags['pb0'], flags['f2b0'])
    if key in _BUILD_CACHE:
        return _BUILD_CACHE[key]
    import concourse.bass as bass
    import concourse.mybir as mybir
    import concourse.tile as tile
    from concourse import bacc
    from concourse.masks import make_identity
    from contextlib import ExitStack

    F32 = mybir.dt.float32
    BF16 = mybir.dt.bfloat16
    AF = mybir.ActivationFunctionType
    ALU = mybir.AluOpType
    AX = mybir.AxisListType
    D = _DEPTH

    nc = bacc.Bacc("TRN2", target_bir_lowering=False, debug=False, num_devices=_NCORES)

    dr = {}
    def din(name, shape, dt):
        dr[name] = nc.dram_tensor(name, list(shape), dt, kind="ExternalInput").ap()
    din('xT', (E_DIM, L), BF16)
    din('decw', (E_DIM, C), BF16)
    din('posb', (L, C), F32)
    din('wqk', (D, C, 2 * C), BF16)
    din('bqk', (D, P, 8), F32)
    din('wvp', (D, C, VW), BF16)
    din('vb', (D, P, VW), F32)
    din('ee', (D, 4, 4, P, 256), BF16)
    din('wp', (D, C, C), BF16)
    din('pb', (D, P, C), F32)
    din('f1', (D, C, FH), BF16)
    din('f1b', (D, P, FH // P), F32)
    din('f2', (D, FH, C), BF16)
    din('f2b', (D, P, C), F32)
    din('nfw', (P, KC), F32)
    din('nfb', (P, KC), F32)
    din('pw', (C, N_E), BF16)
    outT = nc.dram_tensor("outT", [N_E, L], BF16, kind="ExternalOutput").ap()

    with tile.TileContext(nc) as tc, ExitStack() as ES:
        cst = ES.enter_context(tc.tile_pool(name="cst", bufs=1))
        ident_b = None
        if _TP == 'pe':
            ident_f = cst.tile([P, P], F32)
            make_identity(nc, ident_f)
            ident_b = cst.tile([P, P], BF16)
            nc.scalar.copy(ident_b[:], ident_f[:])
        eps_t = cst.tile([P, 1], F32)
        nc.vector.memset(eps_t[:], 1e-5)

        xp = ES.enter_context(tc.tile_pool(name="xp", bufs=1))
        hp = ES.enter_context(tc.tile_pool(name="hp", bufs=2))
        hTp = ES.enter_context(tc.tile_pool(name="hTp", bufs=2))
        qkp = ES.enter_context(tc.tile_pool(name="qkp", bufs=1))
        vp = ES.enter_context(tc.tile_pool(name="vp", bufs=1))
        attp = ES.enter_context(tc.tile_pool(name="attp", bufs=1))
        ptp = ES.enter_context(tc.tile_pool(name="ptp", bufs=2))
        eep = ES.enter_context(tc.tile_pool(name="eep", bufs=2))
        stp = ES.enter_context(tc.tile_pool(name="stp", bufs=2))
        recp = ES.enter_context(tc.tile_pool(name="recp", bufs=2))
        wqkp = ES.enter_context(tc.tile_pool(name="wqkp", bufs=2))
        wvpp = ES.enter_context(tc.tile_pool(name="wvpp", bufs=2))
        wpp = ES.enter_context(tc.tile_pool(name="wpp", bufs=2))
        f1p = ES.enter_context(tc.tile_pool(name="f1p", bufs=3))
        f2p = ES.enter_context(tc.tile_pool(name="f2p", bufs=3))
        gp = ES.enter_context(tc.tile_pool(name="gp", bufs=3))
        bp = ES.enter_context(tc.tile_pool(name="bp", bufs=2))
        outp = ES.enter_context(tc.tile_pool(name="outp", bufs=4))
        pwp = ES.enter_context(tc.tile_pool(name="pwp", bufs=3))

        x = xp.tile([P, NT, C], F32)

        # ---------- dec: x = xT.T @ decw + (pos + dec_b) ----------
        with tc.tile_pool(name="decp", bufs=1) as decp, \
             tc.tile_pool(name="dps", bufs=2, space="PSUM") as dps:
            xT_sb = decp.tile([P, KE, L], BF16)
            nc.sync.dma_start(xT_sb[:], dr['xT'].rearrange("(k p) t -> p k t", p=P))
            decw_sb = decp.tile([P, KE, C], BF16)
            nc.sync.dma_start(decw_sb[:], dr['decw'].rearrange("(k p) c -> p k c", p=P))
            for tt in range(NT):
                pos_t = decp.tile([P, C], F32, name="pos_t", tag="pos", bufs=2)
                nc.sync.dma_start(pos_t[:], dr['posb'][tt * P:(tt + 1) * P, :])
                ps = dps.tile([P, C], F32)
                for kk in range(KE):
                    nc.tensor.matmul(ps[:], xT_sb[:, kk, tt * P:(tt + 1) * P],
                                     decw_sb[:, kk, :], start=(kk == 0), stop=(kk == KE - 1))
                nc.vector.tensor_add(x[:, tt], ps[:], pos_t[:])

        # ---------- layer-norm: h = (x - mean) * rstd  (bf16 out) ----------
        def layernorm(dst, src, pfx):
            stats = stp.tile([P, NT, 6], F32, name=f"{pfx}st", tag=f"{pfx}st")
            mv = stp.tile([P, NT, 2], F32, name=f"{pfx}mv", tag=f"{pfx}mv")
            for tt in range(NT):
                nc.vector.bn_stats(stats[:, tt], src[:, tt])
                nc.vector.bn_aggr(mv[:, tt], stats[:, tt])
            # rstd = exp(-0.5*ln(var+eps)); nb = -mean*rstd
            lnv = stp.tile([P, NT], F32, name=f"{pfx}ln", tag=f"{pfx}ln")
            nc.scalar.activation(lnv[:], mv[:, :, 1], AF.Ln, bias=eps_t[:])
            rstd = stp.tile([P, NT], F32, name=f"{pfx}rs", tag=f"{pfx}rs")
            nc.scalar.activation(rstd[:], lnv[:], AF.Exp, scale=-0.5)
            nb = stp.tile([P, NT], F32, name=f"{pfx}nb", tag=f"{pfx}nb")
            nc.vector.tensor_mul(nb[:], mv[:, :, 0], rstd[:])
            nc.vector.tensor_scalar_mul(nb[:], nb[:], -1.0)
            for tt in range(NT):
                nc.scalar.activation(dst[:, tt], src[:, tt], AF.Identity,
                                     bias=nb[:, tt:tt + 1], scale=rstd[:, tt:tt + 1])

        # transpose token-major [P, NT, C] bf16 -> C-major [P, KC, L] bf16
        def transpose_to(hT, src, tpool=None):
            if _TP == 'dma':
                for ct in range(KC):
                    for tt in range(NT):
                        nc.sync.dma_start_transpose(
                            hT[:, ct, tt * P:(tt + 1) * P],
                            src[:, tt, ct * P:(ct + 1) * P])
            else:
                for ct in range(KC):
                    for g4 in range(2):
                        tps = tpool.tile([P, 4, P], BF16, name="tp")
                        for q in range(4):
                            tt = g4 * 4 + q
                            nc.tensor.transpose(tps[:, q], src[:, tt, ct * P:(ct + 1) * P],
                                                ident_b[:])
                        nc.scalar.copy(hT[:, ct, g4 * 512:(g4 + 1) * 512],
                                       tps[:].rearrange("p a b -> p (a b)"))

        # shift permute in hT space (window-major tokens), DVE+GpSimd copies.
        # fwd: dstT(B)[RB] = srcT(A)[RA]; else dstT(A)[RA] = srcT(B)[RB]
        # ct-outer so each k-slice completes early for downstream matmuls.
        def permute(dstT, srcT, fwd):
            sv = srcT[:].rearrange("p k (a b i j) -> p k a b i j", a=G, b=G, i=WS, j=WS)
            dv = dstT[:].rearrange("p k (a b i j) -> p k a b i j", a=G, b=G, i=WS, j=WS)
            for ct in range(KC):
                n = 0
                for qa in range(2):
                    di = slice(0, 4) if qa == 0 else slice(4, 8)
                    si = slice(4, 8) if qa == 0 else slice(0, 4)
                    for qb in range(2):
                        dj = slice(0, 4) if qb == 0 else slice(4, 8)
                        sj = slice(4, 8) if qb == 0 else slice(0, 4)
                        if qb == 0:
                            bpairs = [(slice(0, G), slice(0, G))]
                        else:
                            bpairs = [(slice(0, G - 1), slice(1, G)),
                                      (slice(G - 1, G), slice(0, 1))]
                        for a in range(G):
                            sa = (a + qa) % G
                            for db, sb_ in bpairs:
                                eng = (nc.gpsimd, nc.vector)[n % 2]
                                n += 1
                                if fwd:
                                    eng.tensor_copy(dv[:, ct, a, db, di, dj],
                                                    sv[:, ct, sa, sb_, si, sj])
                                else:
                                    eng.tensor_copy(dv[:, ct, sa, sb_, si, sj],
                                                    sv[:, ct, a, db, di, dj])

        qkT = qkp.tile([P, 8, L], BF16)
        v_aug = vp.tile([P, NT, VW], BF16)
        att = attp.tile([P, NT, C], BF16)

        # ---------- layers ----------
        for i in range(D):
            shift = (i % 2) == 1
            if _STAGE < 1:
                continue
            # LN1 + transpose (+ shift permute)
            h = hp.tile([P, NT, C], BF16, name="h")
            layernorm(h, x, "l1")
            if _STAGE < 2:
                continue
            with tc.tile_pool(name="tp1", bufs=2, space="PSUM") as tpool:
                hT_A = hTp.tile([P, KC, L], BF16, name="hT")
                transpose_to(hT_A, h, tpool)
            if shift:
                hT = hTp.tile([P, KC, L], BF16, name="hT")
                permute(hT, hT_A, True)
            else:
                hT = hT_A
            if _STAGE < 3:
                continue

            # ---- qk + v ----
            wqk_sb = wqkp.tile([P, KC, 2 * C], BF16, name="wqk")
            nc.sync.dma_start(wqk_sb[:], dr['wqk'][i].rearrange("(k p) m -> p k m", p=P))
            wvp_sb = wvpp.tile([P, KC, VW], BF16, name="wvp")
            nc.sync.dma_start(wvp_sb[:], dr['wvp'][i].rearrange("(k p) m -> p k m", p=P))
            vb_sb = bp.tile([P, VW], F32, name="vb", tag="vb")
            nc.sync.dma_start(vb_sb[:], dr['vb'][i])
            bqk_sb = None
            if not flags['bqk0']:
                bqk_sb = bp.tile([P, 8], F32, name="bqk", tag="bqk")
                nc.sync.dma_start(bqk_sb[:], dr['bqk'][i])
            with tc.tile_pool(name="qkps", bufs=3, space="PSUM") as qkps, \
                 tc.tile_pool(name="vps", bufs=2, space="PSUM") as vps:
                for tc2 in range(2):
                    for mo in range(8):
                        ps = qkps.tile([P, C], F32, name="qkmm")
                        for kk in range(KC):
                            nc.tensor.matmul(ps[:], wqk_sb[:, kk, mo * P:(mo + 1) * P],
                                             hT[:, kk, tc2 * 512:(tc2 + 1) * 512],
                                             start=(kk == 0), stop=(kk == KC - 1))
                        if flags['bqk0']:
                            nc.scalar.copy(qkT[:, mo, tc2 * 512:(tc2 + 1) * 512], ps[:])
                        else:
                            nc.scalar.activation(qkT[:, mo, tc2 * 512:(tc2 + 1) * 512],
                                                 ps[:], AF.Identity,
                                                 bias=bqk_sb[:, mo:mo + 1])
                for tt in range(NT):
                    ps = vps.tile([P, VW], F32, name="vmm")
                    for kk in range(KC):
                        nc.tensor.matmul(ps[:, 0:512], hT[:, kk, tt * P:(tt + 1) * P],
                                         wvp_sb[:, kk, 0:512], start=(kk == 0),
                                         stop=(kk == KC - 1), skip_group_check=True)
                        nc.tensor.matmul(ps[:, 512:VW], hT[:, kk, tt * P:(tt + 1) * P],
                                         wvp_sb[:, kk, 512:VW], start=(kk == 0),
                                         stop=(kk == KC - 1), skip_group_check=True)
                    nc.vector.tensor_add(v_aug[:, tt], ps[:], vb_sb[:])

            if _STAGE < 4:
                continue
            # ---- attention ----
            # EE tiles for this layer
            pts_needed = sorted(set(_WP2_PT)) if shift else [0]
            ee_sb = {}
            for pt in pts_needed:
                for rg in range(4):
                    t = eep.tile([P, 256], BF16, name=f"ee{pt}_{rg}", tag=f"ee{pt}_{rg}")
                    nc.sync.dma_start(t[:], dr['ee'][i, pt, rg])
                    ee_sb[(pt, rg)] = t
            with tc.tile_pool(name="sps", bufs=1, space="PSUM") as sps, \
                 tc.tile_pool(name="avps", bufs=2, space="PSUM") as avps:
                for wp2 in range(NT):
                    pt = _WP2_PT[wp2] if shift else 0
                    # S: one full PSUM bank per rg (row-group); MMs from different
                    # row-groups must not share a bank. rg-interleaved emission so
                    # LDWs pull ahead across row groups.
                    sgs = [sps.tile([P, 8, NW], F32, name=f"s{rg}", tag=f"s{rg}")
                           for rg in range(4)]
                    for j in range(8):
                        hi = j % 4
                        w01 = j // 4
                        wc = slice((wp2 * 2 + w01) * NW, (wp2 * 2 + w01 + 1) * NW)
                        for rg in range(4):
                            nc.tensor.matmul(
                                sgs[rg][w01 * NW:(w01 + 1) * NW, hi, :],
                                qkT[rg * HD:(rg + 1) * HD, 4 + hi, wc],
                                qkT[rg * HD:(rg + 1) * HD, hi, wc],
                                start=True, stop=True, skip_group_check=True,
                                tile_position=(rg * HD, w01 * NW))
                    if _ATT < 1:
                        continue
                    ptg = []
                    for rg in range(4):
                        er = ptp.tile([P, 4, NW], BF16, name=f"er{rg}", tag=f"er{rg}")
                        nc.scalar.activation(er[:].rearrange("p a b -> p (a b)"),
                                             sgs[rg][:, 0:4, :].rearrange("p a b -> p (a b)"),
                                             AF.Exp)
                        pt_t = ptp.tile([P, 4, NW], BF16, name=f"pt{rg}", tag=f"pt{rg}")
                        nc.vector.tensor_mul(pt_t[:].rearrange("p a b -> p (a b)"),
                                             er[:].rearrange("p a b -> p (a b)"),
                                             ee_sb[(pt, rg)][:])
                        ptg.append(pt_t)
                    if _ATT < 2:
                        continue
                    avs = [avps.tile([P, 8, NW], F32, name=f"av{hs}", tag=f"av{hs}")
                           for hs in range(2)]
                    for s in range(8):
                        hi0 = (s // 4) % 2
                        rg = s % 4
                        for hs in range(2):
                            hi = 2 * hs + hi0
                            h_ = hi * 4 + rg
                            for w01 in range(2):
                                rows = slice(w01 * NW, (w01 + 1) * NW)
                                nc.tensor.matmul(
                                    avs[hs][rows, s, 0:VBLK], ptg[rg][rows, hi, :],
                                    v_aug[rows, wp2, h_ * VBLK:(h_ + 1) * VBLK],
                                    start=True, stop=True, skip_group_check=True,
                                    tile_position=(w01 * NW, w01 * NW))
                    if _ATT < 3:
                        continue
                    for hs in range(2):
                        for w01 in range(2):
                            rows = slice(w01 * NW, (w01 + 1) * NW)
                            rec = recp.tile([P, 2, 8], F32, name="rec", tag=f"rec{hs}{w01}")
                            nc.vector.reciprocal(rec[rows, w01], avs[hs][rows, :, HD])
                            rb = rec[rows, w01].rearrange("p (a b) -> p a b", b=1) \
                                .to_broadcast((NW, 8, HD))
                            dst = att[rows, wp2, hs * 256:(hs + 1) * 256] \
                                .rearrange("p (a b) -> p a b", b=HD)
                            nc.vector.tensor_mul(dst, avs[hs][rows, :, 0:HD], rb)

            if _STAGE < 5:
                continue
            # ---- attn transpose back (+ inverse shift permute) + proj ----
            with tc.tile_pool(name="tp2", bufs=2, space="PSUM") as tpool:
                aT_B = hTp.tile([P, KC, L], BF16, name="hT")
                transpose_to(aT_B, att, tpool)
            if shift:
                aT = hTp.tile([P, KC, L], BF16, name="hT")
                permute(aT, aT_B, False)
            else:
                aT = aT_B
            wp_sb = wpp.tile([P, KC, C], BF16, name="wp")
            nc.sync.dma_start(wp_sb[:], dr['wp'][i].rearrange("(k p) m -> p k m", p=P))
            if not flags['pb0']:
                pb_sb = bp.tile([P, C], F32, name="pb", tag="pb")
                nc.sync.dma_start(pb_sb[:], dr['pb'][i])
            with tc.tile_pool(name="pjps", bufs=3, space="PSUM") as pjps:
                for tt in range(NT):
                    ps = pjps.tile([P, C], F32, name="pjmm")
                    for kk in range(KC):
                        nc.tensor.matmul(ps[:], aT[:, kk, tt * P:(tt + 1) * P],
                                         wp_sb[:, kk, :], start=(kk == 0), stop=(kk == KC - 1))
                    nc.vector.tensor_add(x[:, tt], ps[:], x[:, tt])
                    if not flags['pb0']:
                        nc.gpsimd.tensor_add(x[:, tt], x[:, tt], pb_sb[:])

            if _STAGE < 6:
                continue
            # ---- LN2 + transpose + MLP ----
            h2 = hp.tile([P, NT, C], BF16, name="h")
            layernorm(h2, x, "l2")
            with tc.tile_pool(name="tp3", bufs=2, space="PSUM") as tpool:
                h2T = hTp.tile([P, KC, L], BF16, name="hT")
                transpose_to(h2T, h2, tpool)
            if _STAGE < 7:
                continue
            f1b_sb = bp.tile([P, FH // P], F32, name="f1b", tag="f1b")
            nc.sync.dma_start(f1b_sb[:], dr['f1b'][i])
            if not flags['f2b0']:
                f2b_sb = bp.tile([P, C], F32, name="f2b", tag="f2b")
                nc.sync.dma_start(f2b_sb[:], dr['f2b'][i])
            with tc.tile_pool(name="f1ps", bufs=3, space="PSUM") as f1ps, \
                 tc.tile_pool(name="fc2ps", bufs=1, space="PSUM") as fc2ps:
                for tc2 in range(2):
                    pso = [fc2ps.tile([P, C], F32, name=f"fc2_{j}", tag=f"fc2_{j}")
                           for j in range(4)]
                    for ho in range(FH // P):
                        f1c = f1p.tile([P, KC, P], BF16, name="f1c")
                        nc.sync.dma_start(f1c[:], dr['f1'][i][:, ho * P:(ho + 1) * P]
                                          .rearrange("(k p) m -> p k m", p=P))
                        f2c = f2p.tile([P, C], BF16, name="f2c")
                        nc.sync.dma_start(f2c[:], dr['f2'][i][ho * P:(ho + 1) * P, :])
                        ps1 = f1ps.tile([P, C], F32, name="f1mm")
                        for kk in range(KC):
                            nc.tensor.matmul(ps1[:], f1c[:, kk, :],
                                             h2T[:, kk, tc2 * 512:(tc2 + 1) * 512],
                                             start=(kk == 0), stop=(kk == KC - 1))
                        g = gp.tile([P, C], BF16, name="g")
                        nc.scalar.activation(g[:], ps1[:], AF.Gelu, bias=f1b_sb[:, ho:ho + 1])
                        for j in range(4):
                            nc.tensor.matmul(pso[j][:], g[:, j * P:(j + 1) * P], f2c[:],
                                             start=(ho == 0), stop=(ho == FH // P - 1))
                    for j in range(4):
                        tt = tc2 * 4 + j
                        nc.vector.tensor_add(x[:, tt], pso[j][:], x[:, tt])
                        if not flags['f2b0']:
                            nc.gpsimd.tensor_add(x[:, tt], x[:, tt], f2b_sb[:])

        # ---------- final LN + gelu + pred ----------
        hf = hp.tile([P, NT, C], BF16, name="h")
        layernorm(hf, x, "lf")
        with tc.tile_pool(name="tpf", bufs=2, space="PSUM") as tpool:
            hfT = hTp.tile([P, KC, L], BF16, name="hT")
            transpose_to(hfT, hf, tpool)
        nfw_sb = bp.tile([P, KC], F32, name="nfw", tag="nfw")
        nc.sync.dma_start(nfw_sb[:], dr['nfw'])
        nfb_sb = bp.tile([P, KC], F32, name="nfb", tag="nfb")
        nc.sync.dma_start(nfb_sb[:], dr['nfb'])
        gT = hTp.tile([P, KC, L], BF16, name="gT", bufs=1)
        for ct in range(KC):
            nc.scalar.activation(gT[:, ct], hfT[:, ct], AF.Gelu,
                                 bias=nfb_sb[:, ct:ct + 1], scale=nfw_sb[:, ct:ct + 1])
        with tc.tile_pool(name="mmpsf", bufs=4, space="PSUM") as mmps:
            for no in range(N_E // P):
                pwc = pwp.tile([P, KC, P], BF16, name="pwc")
                nc.sync.dma_start(pwc[:], dr['pw'][:, no * P:(no + 1) * P]
                                  .rearrange("(k p) m -> p k m", p=P))
                for tc2 in range(2):
                    ps = mmps.tile([P, 512], F32, name="pmm")
                    for kk in range(KC):
                        nc.tensor.matmul(ps[:], pwc[:, kk, :],
                                         gT[:, kk, tc2 * 512:(tc2 + 1) * 512],
                                         start=(kk == 0), stop=(kk == KC - 1))
                    osb = outp.tile([P, 512], BF16, name="osb")
                    if no % 2 == 0:
                        nc.scalar.copy(osb[:], ps[:])
                    else:
                        nc.vector.tensor_copy(osb[:], ps[:])
                    nc.sync.dma_start(outT[no * P:(no + 1) * P, tc2 * 512:(tc2 + 1) * 512],
                                      osb[:])

    nc.compile()
    _BUILD_CACHE[key] = nc
    return nc


LAST_RESULTS = None


def kernel(**inputs):
    global LAST_RESULTS
    from concourse import bass_utils
    sh, xts, pred_b, flags = _prepare(inputs)
    nc = _build(flags)
    in_maps = []
    for c in range(_NCORES):
        m = dict(sh)
        m['xT'] = xts[c % B]
        in_maps.append(m)
    trace = os.environ.get("BT_TRACE", "0") == "1"
    if trace:
        try:
            import antenv.axon_hooks  # noqa: F401
        except ImportError:
            trace = False
    res = bass_utils.run_bass_kernel_spmd(nc, in_maps, core_ids=list(range(_NCORES)),
                                          trace=trace)
    LAST_RESULTS = res
    outs = []
    for c in range(B):
        oT = np.asarray(res.results[c % _NCORES]['outT'], dtype=np.float32)  # [N_E, L]
        o = oT.T[_WM_INV] + pred_b[None, :]        # [L, N_E] raster order
        outs.append(o)
    return np.stack(outs).astype(np.float32)


# revision 15
# speedup vs baseline: 1.2217x; 1.2217x over previous
import os
import numpy as np

# ---- problem constants (hardcoded; kernel.py must be self-contained) ----
IMG, WS, SHIFT = 32, 8, 4
C, HEADS, DEPTH = 512, 16, 24
E_DIM, N_E, B = 256, 8192, 8
L = IMG * IMG            # 1024
NW = WS * WS             # 64 tokens per window
HD = C // HEADS          # 32
NWIN = (IMG // WS) ** 2  # 16
FH = 4 * C               # 2048
P = 128
VBLK = HD + 2            # 34 (32 vals + softmax-denominator col + pad)
VW = HEADS * VBLK        # 544
NT = L // P              # 8 token tiles
KC = C // P              # 4 k-tiles over C
KE = E_DIM // P          # 2 k-tiles over E_DIM
SCALE = HD ** -0.5
G = IMG // WS            # 4 windows per side

_DEPTH = int(os.environ.get("BT_DEPTH", DEPTH))
_NCORES = int(os.environ.get("BT_NCORES", 8))
_TP = os.environ.get("BT_TP", "pe")  # 'dma' or 'pe' transposes
_STAGE = int(os.environ.get("BT_STAGE", "99"))
_ATT = int(os.environ.get("BT_ATT", "3"))


# ---- host-side helpers (mirror reference.py) ----
def _rel_index():
    coords = np.stack(np.meshgrid(np.arange(WS), np.arange(WS), indexing='ij'))
    cf = coords.reshape(2, -1)
    rel = (cf[:, :, None] - cf[:, None, :]).transpose(1, 2, 0)
    rel[:, :, 0] += WS - 1
    rel[:, :, 1] += WS - 1
    rel[:, :, 0] *= 2 * WS - 1
    return rel.sum(-1)  # [NW, NW] int


def _shift_mask():
    img = np.zeros((IMG, IMG), np.float32)
    cnt = 0
    sl = (slice(0, -WS), slice(-WS, -SHIFT), slice(-SHIFT, None))
    for hs in sl:
        for ws_ in sl:
            img[hs, ws_] = cnt
            cnt += 1
    win = img.reshape(IMG // WS, WS, IMG // WS, WS).transpose(0, 2, 1, 3).reshape(-1, NW)
    diff = win[:, None, :] - win[:, :, None]
    return np.where(diff != 0, -100.0, 0.0).astype(np.float32)  # [NWIN, NW, NW]


def _win_perm():
    t = np.arange(L).reshape(IMG, IMG)
    wm = t.reshape(IMG // WS, WS, IMG // WS, WS).transpose(0, 2, 1, 3).reshape(-1)
    inv = np.empty(L, np.int64)
    inv[wm] = np.arange(L)
    return wm, inv  # wm: dev->raster, inv: raster->dev


_WM, _WM_INV = _win_perm()
_REL = _rel_index()
_MASK = _shift_mask()

# representative window index per pattern type for the shifted-layer mask:
#   0: interior (no mask), 1: right-column, 2: bottom-row, 3: corner
_PTYPE_WIN = [0, 3, 12, 15]
# window-pair (wp2) -> pair-type index into EE tiles (shift layers)
#   wp2 = 2a + c ; pair windows (4a+2c, 4a+2c+1)
_WP2_PT = [0 if (wp2 // 2) < 3 else 2 for wp2 in range(NT)]
for _wp2 in range(NT):
    if _wp2 % 2 == 1:
        _WP2_PT[_wp2] += 1
# per pair-type, the window type of (w01=0, w01=1)
_PT_WTYPES = [(0, 0), (0, 1), (2, 2), (2, 3)]
_WTYPE_MASK = {0: np.zeros((NW, NW), np.float32),
               1: None, 2: None, 3: None}


def _wtype_masks():
    m = {0: np.zeros((NW, NW), np.float32)}
    m[1] = _MASK[3]
    m[2] = _MASK[12]
    m[3] = _MASK[15]
    return m


def _prepare(inputs):
    import ml_dtypes
    bf16 = ml_dtypes.bfloat16
    f32 = lambda a: np.ascontiguousarray(a, dtype=np.float32)
    b16 = lambda a: np.ascontiguousarray(np.asarray(a, np.float32).astype(bf16))
    x = np.asarray(inputs['x'], np.float32)
    dec_w = np.asarray(inputs['dec_w'], np.float32)
    dec_b = np.asarray(inputs['dec_b'], np.float32)
    pos = np.asarray(inputs['pos_embed'], np.float32)[0]
    n1w = np.asarray(inputs['n1w'], np.float32)
    n1b = np.asarray(inputs['n1b'], np.float32)
    qkv_w = np.asarray(inputs['qkv_w'], np.float32)
    qkv_b = np.asarray(inputs['qkv_b'], np.float32)
    proj_w = np.asarray(inputs['proj_w'], np.float32)
    proj_b = np.asarray(inputs['proj_b'], np.float32)
    rel_bias = np.asarray(inputs['rel_bias'], np.float32)
    n2w = np.asarray(inputs['n2w'], np.float32)
    n2b = np.asarray(inputs['n2b'], np.float32)
    fc1_w = np.asarray(inputs['fc1_w'], np.float32)
    fc1_b = np.asarray(inputs['fc1_b'], np.float32)
    fc2_w = np.asarray(inputs['fc2_w'], np.float32)
    fc2_b = np.asarray(inputs['fc2_b'], np.float32)
    normf_w = np.asarray(inputs['normf_w'], np.float32)
    normf_b = np.asarray(inputs['normf_b'], np.float32)
    pred_w = np.asarray(inputs['pred_w'], np.float32)
    pred_b = np.asarray(inputs['pred_b'], np.float32)

    D = _DEPTH
    sh = {}
    sh['decw'] = b16(dec_w.T)                       # [E, C]
    sh['posb'] = b16((pos + dec_b[None, :])[_WM])   # [L, C]

    wqk = np.empty((D, C, 2 * C), bf16)
    bqk = np.zeros((D, P, 8), np.float32)
    wvp = np.zeros((D, C, VW), bf16)
    vb = np.zeros((D, P, VW), np.float32)
    ee = np.empty((D, 4, 4, P, 256), bf16)
    wp_ = np.empty((D, C, C), bf16)
    pb = np.empty((D, P, C), np.float32)
    f1 = np.empty((D, C, FH), bf16)
    f1b = np.empty((D, P, FH // P), np.float32)
    f2 = np.empty((D, FH, C), bf16)
    f2b = np.empty((D, P, C), np.float32)

    wtm = _wtype_masks()

    for i in range(D):
        Wm = qkv_w[i] * n1w[i][None, :]
        bm = qkv_w[i] @ n1b[i] + qkv_b[i]
        Wm = Wm.copy()
        bm = bm.copy()
        Wm[:C] *= SCALE
        bm[:C] *= SCALE
        wqk[i] = Wm[:2 * C].T.astype(bf16)
        bqk[i] = np.asarray(bm[:2 * C].reshape(8, P).T, np.float32)
        for h in range(HEADS):
            wvp[i][:, h * VBLK:h * VBLK + HD] = Wm[2 * C + h * HD:2 * C + (h + 1) * HD].T.astype(bf16)
            vb[i][:, h * VBLK:h * VBLK + HD] = bm[2 * C + h * HD:2 * C + (h + 1) * HD][None, :]
            vb[i][:, h * VBLK + HD] = 1.0
        # multiplicative attention bias: EE[pt, hs][p = w01*64 + tk, s*64 + tq]
        #   = exp(B_h[tq, tk] + mask[wtype][tq, tq->tk])  (transposed into [tk, tq])
        bias = rel_bias[i][_REL]                  # [tq, tk, HEADS]
        shift = (i % 2) == 1
        for pt in range(4):
            wt0, wt1 = _PT_WTYPES[pt]
            for rg in range(4):
                for hi in range(4):
                    h = hi * 4 + rg
                    a = bias[:, :, h].T           # [tk, tq]
                    for w01, wt in ((0, wt0), (1, wt1)):
                        m = wtm[wt].T if shift else wtm[0]
                        e = np.exp(a + m)         # [tk, tq]
                        ee[i, pt, rg, w01 * NW:(w01 + 1) * NW, hi * NW:(hi + 1) * NW] = e.astype(bf16)
        wp_[i] = proj_w[i].T.astype(bf16)
        pb[i] = np.broadcast_to(proj_b[i][None, :], (P, C))
        f1[i] = (fc1_w[i] * n2w[i][None, :]).T.astype(bf16)
        f1b[i] = np.asarray((fc1_w[i] @ n2b[i] + fc1_b[i]).reshape(FH // P, P).T, np.float32)
        f2[i] = fc2_w[i].T.astype(bf16)
        f2b[i] = np.broadcast_to(fc2_b[i][None, :], (P, C))

    sh['wqk'] = wqk
    sh['bqk'] = bqk
    sh['wvp'] = wvp
    sh['vb'] = vb
    sh['ee'] = np.ascontiguousarray(ee)
    sh['wp'] = wp_
    sh['pb'] = pb
    sh['f1'] = f1
    sh['f1b'] = f1b
    sh['f2'] = f2
    sh['f2b'] = f2b
    sh['nfw'] = f32(normf_w.reshape(KC, P).T)     # [P, KC]
    sh['nfb'] = f32(normf_b.reshape(KC, P).T)
    sh['pw'] = b16(pred_w.T)                      # [C, N_E]
    xts = [np.ascontiguousarray(x[c][_WM].T.astype(bf16)) for c in range(B)]
    flags = {
        'bqk0': bool(np.all(bqk == 0.0)),
        'pb0': bool(np.all(pb == 0.0)),
        'f2b0': bool(np.all(f2b == 0.0)),
    }
    return sh, xts, np.asarray(pred_b, np.float32), flags


# ---- device program ----
_BUILD_CACHE = {}


def _build(flags):
    key = (_DEPTH, _TP, _STAGE, _ATT, flags['bqk0'], flags['pb0'], flags['f2b0'])
    if key in _BUILD_CACHE:
        return _BUILD_CACHE[key]
    import concourse.bass as bass
    import concourse.mybir as mybir
    import concourse.tile as tile
    from concourse import bacc
    from concourse.masks import make_identity
    from contextlib import ExitStack

    F32 = mybir.dt.float32
    BF16 = mybir.dt.bfloat16
    AF = mybir.ActivationFunctionType
    ALU = mybir.AluOpType
    AX = mybir.AxisListType
    D = _DEPTH

    nc = bacc.Bacc("TRN2", target_bir_lowering=False, debug=False, num_devices=_NCORES)

    dr = {}
    def din(name, shape, dt):
        dr[name] = nc.dram_tensor(name, list(shape), dt, kind="ExternalInput").ap()
    din('xT', (E_DIM, L), BF16)
    din('decw', (E_DIM, C), BF16)
    din('posb', (L, C), BF16)
    din('wqk', (D, C, 2 * C), BF16)
    din('bqk', (D, P, 8), F32)
    din('wvp', (D, C, VW), BF16)
    din('vb', (D, P, VW), F32)
    din('ee', (D, 4, 4, P, 256), BF16)
    din('wp', (D, C, C), BF16)
    din('pb', (D, P, C), F32)
    din('f1', (D, C, FH), BF16)
    din('f1b', (D, P, FH // P), F32)
    din('f2', (D, FH, C), BF16)
    din('f2b', (D, P, C), F32)
    din('nfw', (P, KC), F32)
    din('nfb', (P, KC), F32)
    din('pw', (C, N_E), BF16)
    outT = nc.dram_tensor("outT", [N_E, L], BF16, kind="ExternalOutput").ap()

    with tile.TileContext(nc) as tc, ExitStack() as ES:
        cst = ES.enter_context(tc.tile_pool(name="cst", bufs=1))
        ident_b = None
        if _TP == 'pe':
            ident_f = cst.tile([P, P], F32)
            make_identity(nc, ident_f)
            ident_b = cst.tile([P, P], BF16)
            nc.scalar.copy(ident_b[:], ident_f[:])
        eps_t = cst.tile([P, 1], F32)
        nc.vector.memset(eps_t[:], 1e-5)

        xp = ES.enter_context(tc.tile_pool(name="xp", bufs=1))
        hp = ES.enter_context(tc.tile_pool(name="hp", bufs=2))
        hTp = ES.enter_context(tc.tile_pool(name="hTp", bufs=2))
        qkp = ES.enter_context(tc.tile_pool(name="qkp", bufs=1))
        vp = ES.enter_context(tc.tile_pool(name="vp", bufs=1))
        attp = ES.enter_context(tc.tile_pool(name="attp", bufs=1))
        ptp = ES.enter_context(tc.tile_pool(name="ptp", bufs=2))
        eep = ES.enter_context(tc.tile_pool(name="eep", bufs=1))
        stp = ES.enter_context(tc.tile_pool(name="stp", bufs=2))
        recp = ES.enter_context(tc.tile_pool(name="recp", bufs=2))
        wqkp = ES.enter_context(tc.tile_pool(name="wqkp", bufs=2))
        wvpp = ES.enter_context(tc.tile_pool(name="wvpp", bufs=2))
        wpp = ES.enter_context(tc.tile_pool(name="wpp", bufs=2))
        f1p = ES.enter_context(tc.tile_pool(name="f1p", bufs=2))
        f2p = ES.enter_context(tc.tile_pool(name="f2p", bufs=2))
        gp = ES.enter_context(tc.tile_pool(name="gp", bufs=3))
        bp = ES.enter_context(tc.tile_pool(name="bp", bufs=2))
        outp = ES.enter_context(tc.tile_pool(name="outp", bufs=2))
        pwp = ES.enter_context(tc.tile_pool(name="pwp", bufs=2))

        x = xp.tile([P, NT, C], F32)

        # ---------- dec: x = xT.T @ decw + (pos + dec_b) ----------
        with tc.tile_pool(name="decp", bufs=1) as decp, \
             tc.tile_pool(name="dps", bufs=2, space="PSUM") as dps:
            xT_sb = decp.tile([P, KE, L], BF16)
            nc.sync.dma_start(xT_sb[:], dr['xT'].rearrange("(k p) t -> p k t", p=P))
            decw_sb = decp.tile([P, KE, C], BF16)
            nc.sync.dma_start(decw_sb[:], dr['decw'].rearrange("(k p) c -> p k c", p=P))
            pos_t = decp.tile([P, NT, C], BF16, name="pos_t", tag="pos")
            nc.sync.dma_start(pos_t[:], dr['posb'].rearrange("(t p) c -> p t c", p=P))
            for tt in range(NT):
                ps = dps.tile([P, C], F32)
                for kk in range(KE):
                    nc.tensor.matmul(ps[:], xT_sb[:, kk, tt * P:(tt + 1) * P],
                                     decw_sb[:, kk, :], start=(kk == 0), stop=(kk == KE - 1))
                nc.vector.tensor_add(x[:, tt], ps[:], pos_t[:, tt])

        # ---------- layer-norm: h = (x - mean) * rstd  (bf16 out) ----------
        def layernorm(dst, src, pfx):
            stats = stp.tile([P, NT, 6], F32, name=f"{pfx}st", tag=f"{pfx}st")
            mv = stp.tile([P, NT, 2], F32, name=f"{pfx}mv", tag=f"{pfx}mv")
            for tt in range(NT):
                nc.vector.bn_stats(stats[:, tt], src[:, tt])
                nc.vector.bn_aggr(mv[:, tt], stats[:, tt])
            # rstd = exp(-0.5*ln(var+eps)); nb = -mean*rstd
            lnv = stp.tile([P, NT], F32, name=f"{pfx}ln", tag=f"{pfx}ln")
            nc.scalar.activation(lnv[:], mv[:, :, 1], AF.Ln, bias=eps_t[:])
            rstd = stp.tile([P, NT], F32, name=f"{pfx}rs", tag=f"{pfx}rs")
            nc.scalar.activation(rstd[:], lnv[:], AF.Exp, scale=-0.5)
            nb = stp.tile([P, NT], F32, name=f"{pfx}nb", tag=f"{pfx}nb")
            nc.vector.tensor_mul(nb[:], mv[:, :, 0], rstd[:])
            nc.vector.tensor_scalar_mul(nb[:], nb[:], -1.0)
            for tt in range(NT):
                nc.scalar.activation(dst[:, tt], src[:, tt], AF.Identity,
                                     bias=nb[:, tt:tt + 1], scale=rstd[:, tt:tt + 1])

        # transpose token-major [P, NT, C] bf16 -> C-major [P, KC, L] bf16
        def transpose_to(hT, src, tpool=None):
            if _TP == 'dma':
                for ct in range(KC):
                    for tt in range(NT):
                        nc.sync.dma_start_transpose(
                            hT[:, ct, tt * P:(tt + 1) * P],
                            src[:, tt, ct * P:(ct + 1) * P])
            else:
                for ct in range(KC):
                    for g4 in range(2):
                        tps = tpool.tile([P, 4, P], BF16, name="tp")
                        for q in range(4):
                            tt = g4 * 4 + q
                            nc.tensor.transpose(tps[:, q], src[:, tt, ct * P:(ct + 1) * P],
                                                ident_b[:])
                        nc.scalar.copy(hT[:, ct, g4 * 512:(g4 + 1) * 512],
                                       tps[:].rearrange("p a b -> p (a b)"))

        # shift permute in hT space (window-major tokens), DVE+GpSimd copies.
        # fwd: dstT(B)[RB] = srcT(A)[RA]; else dstT(A)[RA] = srcT(B)[RB]
        # ct-outer so each k-slice completes early for downstream matmuls.
        def permute(dstT, srcT, fwd):
            sv = srcT[:].rearrange("p k (a b i j) -> p k a b i j", a=G, b=G, i=WS, j=WS)
            dv = dstT[:].rearrange("p k (a b i j) -> p k a b i j", a=G, b=G, i=WS, j=WS)
            for ct in range(KC):
                n = 0
                for qa in range(2):
                    di = slice(0, 4) if qa == 0 else slice(4, 8)
                    si = slice(4, 8) if qa == 0 else slice(0, 4)
                    for qb in range(2):
                        dj = slice(0, 4) if qb == 0 else slice(4, 8)
                        sj = slice(4, 8) if qb == 0 else slice(0, 4)
                        if qb == 0:
                            bpairs = [(slice(0, G), slice(0, G))]
                        else:
                            bpairs = [(slice(0, G - 1), slice(1, G)),
                                      (slice(G - 1, G), slice(0, 1))]
                        for a in range(G):
                            sa = (a + qa) % G
                            for db, sb_ in bpairs:
                                eng = (nc.gpsimd, nc.vector)[n % 2]
                                n += 1
                                if fwd:
                                    eng.tensor_copy(dv[:, ct, a, db, di, dj],
                                                    sv[:, ct, sa, sb_, si, sj])
                                else:
                                    eng.tensor_copy(dv[:, ct, sa, sb_, si, sj],
                                                    sv[:, ct, a, db, di, dj])

        qkT = qkp.tile([P, 8, L], BF16)
        v_aug = vp.tile([P, NT, VW], BF16)
        att = attp.tile([P, NT, C], BF16)

        # ---------- layers ----------
        for i in range(D):
            shift = (i % 2) == 1
            if _STAGE < 1:
                continue
            # LN1 + transpose (+ shift permute)
            h = hp.tile([P, NT, C], BF16, name="h")
            layernorm(h, x, "l1")
            if _STAGE < 2:
                continue
            with tc.tile_pool(name="tp1", bufs=2, space="PSUM") as tpool:
                hT_A = hTp.tile([P, KC, L], BF16, name="hT")
                transpose_to(hT_A, h, tpool)
            if shift:
                hT = hTp.tile([P, KC, L], BF16, name="hT")
                permute(hT, hT_A, True)
            else:
                hT = hT_A
            if _STAGE < 3:
                continue

            # ---- qk + v ----
            wqk_sb = wqkp.tile([P, KC, 2 * C], BF16, name="wqk")
            nc.sync.dma_start(wqk_sb[:], dr['wqk'][i].rearrange("(k p) m -> p k m", p=P))
            wvp_sb = wvpp.tile([P, KC, VW], BF16, name="wvp")
            nc.sync.dma_start(wvp_sb[:], dr['wvp'][i].rearrange("(k p) m -> p k m", p=P))
            vb_sb = bp.tile([P, VW], F32, name="vb", tag="vb")
            nc.sync.dma_start(vb_sb[:], dr['vb'][i])
            bqk_sb = None
            if not flags['bqk0']:
                bqk_sb = bp.tile([P, 8], F32, name="bqk", tag="bqk")
                nc.sync.dma_start(bqk_sb[:], dr['bqk'][i])
            with tc.tile_pool(name="qkps", bufs=3, space="PSUM") as qkps, \
                 tc.tile_pool(name="vps", bufs=2, space="PSUM") as vps:
                for tc2 in range(2):
                    for mo in range(8):
                        ps = qkps.tile([P, C], F32, name="qkmm")
                        for kk in range(KC):
                            nc.tensor.matmul(ps[:], wqk_sb[:, kk, mo * P:(mo + 1) * P],
                                             hT[:, kk, tc2 * 512:(tc2 + 1) * 512],
                                             start=(kk == 0), stop=(kk == KC - 1))
                        if flags['bqk0']:
                            nc.scalar.copy(qkT[:, mo, tc2 * 512:(tc2 + 1) * 512], ps[:])
                        else:
                            nc.scalar.activation(qkT[:, mo, tc2 * 512:(tc2 + 1) * 512],
                                                 ps[:], AF.Identity,
                                                 bias=bqk_sb[:, mo:mo + 1])
                for tt in range(NT):
                    ps = vps.tile([P, VW], F32, name="vmm")
                    for kk in range(KC):
                        nc.tensor.matmul(ps[:, 0:512], hT[:, kk, tt * P:(tt + 1) * P],
                                         wvp_sb[:, kk, 0:512], start=(kk == 0),
                                         stop=(kk == KC - 1), skip_group_check=True)
                        nc.tensor.matmul(ps[:, 512:VW], hT[:, kk, tt * P:(tt + 1) * P],
                                         wvp_sb[:, kk, 512:VW], start=(kk == 0),
                                         stop=(kk == KC - 1), skip_group_check=True)
                    nc.vector.tensor_add(v_aug[:, tt], ps[:], vb_sb[:])

            if _STAGE < 4:
                continue
            # ---- attention ----
            # EE tiles for this layer
            pts_needed = sorted(set(_WP2_PT)) if shift else [0]
            ee_sb = {}
            for pt in pts_needed:
                t = eep.tile([P, 4, 256], BF16, name=f"ee{pt}", tag=f"ee{pt}")
                nc.sync.dma_start(t[:], dr['ee'][i, pt].rearrange("r p c -> p r c"))
                for rg in range(4):
                    ee_sb[(pt, rg)] = t[:, rg]
            with tc.tile_pool(name="sps", bufs=1, space="PSUM") as sps, \
                 tc.tile_pool(name="avps", bufs=2, space="PSUM") as avps:
                for wp2 in range(NT):
                    pt = _WP2_PT[wp2] if shift else 0
                    # S: one full PSUM bank per rg (row-group); MMs from different
                    # row-groups must not share a bank. rg-interleaved emission so
                    # LDWs pull ahead across row groups.
                    sgs = [sps.tile([P, 8, NW], F32, name=f"s{rg}", tag=f"s{rg}")
                           for rg in range(4)]
                    for j in range(8):
                        hi = j % 4
                        w01 = j // 4
                        wc = slice((wp2 * 2 + w01) * NW, (wp2 * 2 + w01 + 1) * NW)
                        for rg in range(4):
                            nc.tensor.matmul(
                                sgs[rg][w01 * NW:(w01 + 1) * NW, hi, :],
                                qkT[rg * HD:(rg + 1) * HD, 4 + hi, wc],
                                qkT[rg * HD:(rg + 1) * HD, hi, wc],
                                start=True, stop=True, skip_group_check=True,
                                tile_position=(rg * HD, w01 * NW))
                    if _ATT < 1:
                        continue
                    ptg = []
                    for rg in range(4):
                        er = ptp.tile([P, 4, NW], BF16, name=f"er{rg}", tag=f"er{rg}")
                        nc.scalar.activation(er[:].rearrange("p a b -> p (a b)"),
                                             sgs[rg][:, 0:4, :].rearrange("p a b -> p (a b)"),
                                             AF.Exp)
                        pt_t = ptp.tile([P, 4, NW], BF16, name=f"pt{rg}", tag=f"pt{rg}")
                        nc.vector.tensor_mul(pt_t[:].rearrange("p a b -> p (a b)"),
                                             er[:].rearrange("p a b -> p (a b)"),
                                             ee_sb[(pt, rg)])
                        ptg.append(pt_t)
                    if _ATT < 2:
                        continue
                    avs = [avps.tile([P, 8, NW], F32, name=f"av{hs}", tag=f"av{hs}")
                           for hs in range(2)]
                    for s in range(8):
                        hi0 = (s // 4) % 2
                        rg = s % 4
                        for hs in range(2):
                            hi = 2 * hs + hi0
                            h_ = hi * 4 + rg
                            for w01 in range(2):
                                rows = slice(w01 * NW, (w01 + 1) * NW)
                                nc.tensor.matmul(
                                    avs[hs][rows, s, 0:VBLK], ptg[rg][rows, hi, :],
                                    v_aug[rows, wp2, h_ * VBLK:(h_ + 1) * VBLK],
                                    start=True, stop=True, skip_group_check=True,
                                    tile_position=(w01 * NW, w01 * NW))
                    if _ATT < 3:
                        continue
                    for hs in range(2):
                        for w01 in range(2):
                            rows = slice(w01 * NW, (w01 + 1) * NW)
                            rec = recp.tile([P, 2, 8], F32, name="rec", tag=f"rec{hs}{w01}")
                            nc.vector.reciprocal(rec[rows, w01], avs[hs][rows, :, HD])
                            rb = rec[rows, w01].rearrange("p (a b) -> p a b", b=1) \
                                .to_broadcast((NW, 8, HD))
                            dst = att[rows, wp2, hs * 256:(hs + 1) * 256] \
                                .rearrange("p (a b) -> p a b", b=HD)
                            nc.vector.tensor_mul(dst, avs[hs][rows, :, 0:HD], rb)

            if _STAGE < 5:
                continue
            # ---- attn transpose back (+ inverse shift permute) + proj ----
            with tc.tile_pool(name="tp2", bufs=2, space="PSUM") as tpool:
                aT_B = hTp.tile([P, KC, L], BF16, name="hT")
                transpose_to(aT_B, att, tpool)
            if shift:
                aT = hTp.tile([P, KC, L], BF16, name="hT")
                permute(aT, aT_B, False)
            else:
                aT = aT_B
            wp_sb = wpp.tile([P, KC, C], BF16, name="wp")
            nc.sync.dma_start(wp_sb[:], dr['wp'][i].rearrange("(k p) m -> p k m", p=P))
            if not flags['pb0']:
                pb_sb = bp.tile([P, C], F32, name="pb", tag="pb")
                nc.sync.dma_start(pb_sb[:], dr['pb'][i])
            with tc.tile_pool(name="pjps", bufs=3, space="PSUM") as pjps:
                for tt in range(NT):
                    ps = pjps.tile([P, C], F32, name="pjmm")
                    for kk in range(KC):
                        nc.tensor.matmul(ps[:], aT[:, kk, tt * P:(tt + 1) * P],
                                         wp_sb[:, kk, :], start=(kk == 0), stop=(kk == KC - 1))
                    nc.vector.tensor_add(x[:, tt], ps[:], x[:, tt])
                    if not flags['pb0']:
                        nc.gpsimd.tensor_add(x[:, tt], x[:, tt], pb_sb[:])

            if _STAGE < 6:
                continue
            # ---- LN2 + transpose + MLP ----
            h2 = hp.tile([P, NT, C], BF16, name="h")
            layernorm(h2, x, "l2")
            with tc.tile_pool(name="tp3", bufs=2, space="PSUM") as tpool:
                h2T = hTp.tile([P, KC, L], BF16, name="hT")
                transpose_to(h2T, h2, tpool)
            if _STAGE < 7:
                continue
            f1b_sb = bp.tile([P, FH // P], F32, name="f1b", tag="f1b")
            nc.sync.dma_start(f1b_sb[:], dr['f1b'][i])
            if not flags['f2b0']:
                f2b_sb = bp.tile([P, C], F32, name="f2b", tag="f2b")
                nc.sync.dma_start(f2b_sb[:], dr['f2b'][i])
            with tc.tile_pool(name="f1ps", bufs=3, space="PSUM") as f1ps, \
                 tc.tile_pool(name="fc2ps", bufs=1, space="PSUM") as fc2ps:
                for tc2 in range(2):
                    pso = [fc2ps.tile([P, C], F32, name=f"fc2_{j}", tag=f"fc2_{j}")
                           for j in range(4)]
                    for q in range(4):
                        f1c = f1p.tile([P, KC, 4 * P], BF16, name="f1c")
                        nc.sync.dma_start(f1c[:], dr['f1'][i][:, q * 512:(q + 1) * 512]
                                          .rearrange("(k p) m -> p k m", p=P))
                        f2c = f2p.tile([P, 4, C], BF16, name="f2c")
                        nc.sync.dma_start(f2c[:], dr['f2'][i][q * 512:(q + 1) * 512, :]
                                          .rearrange("(o p) c -> p o c", p=P))
                        for oq in range(4):
                            ho = q * 4 + oq
                            ps1 = f1ps.tile([P, C], F32, name="f1mm")
                            for kk in range(KC):
                                nc.tensor.matmul(ps1[:], f1c[:, kk, oq * P:(oq + 1) * P],
                                                 h2T[:, kk, tc2 * 512:(tc2 + 1) * 512],
                                                 start=(kk == 0), stop=(kk == KC - 1))
                            g = gp.tile([P, C], BF16, name="g")
                            nc.scalar.activation(g[:], ps1[:], AF.Gelu, bias=f1b_sb[:, ho:ho + 1])
                            for j in range(4):
                                nc.tensor.matmul(pso[j][:], g[:, j * P:(j + 1) * P],
                                                 f2c[:, oq, :],
                                                 start=(ho == 0), stop=(ho == FH // P - 1))
                    for j in range(4):
                        tt = tc2 * 4 + j
                        nc.vector.tensor_add(x[:, tt], pso[j][:], x[:, tt])
                        if not flags['f2b0']:
                            nc.gpsimd.tensor_add(x[:, tt], x[:, tt], f2b_sb[:])

        # ---------- final LN + gelu + pred ----------
        hf = hp.tile([P, NT, C], BF16, name="h")
        layernorm(hf, x, "lf")
        with tc.tile_pool(name="tpf", bufs=2, space="PSUM") as tpool:
            hfT = hTp.tile([P, KC, L], BF16, name="hT")
            transpose_to(hfT, hf, tpool)
        nfw_sb = bp.tile([P, KC], F32, name="nfw", tag="nfw")
        nc.sync.dma_start(nfw_sb[:], dr['nfw'])
        nfb_sb = bp.tile([P, KC], F32, name="nfb", tag="nfb")
        nc.sync.dma_start(nfb_sb[:], dr['nfb'])
        gT = hTp.tile([P, KC, L], BF16, name="gT", bufs=1)
        for ct in range(KC):
            nc.scalar.activation(gT[:, ct], hfT[:, ct], AF.Gelu,
                                 bias=nfb_sb[:, ct:ct + 1], scale=nfw_sb[:, ct:ct + 1])
        with tc.tile_pool(name="mmpsf", bufs=4, space="PSUM") as mmps:
            for n4 in range(N_E // (4 * P)):
                pwc = pwp.tile([P, KC, 4 * P], BF16, name="pwc")
                nc.sync.dma_start(pwc[:], dr['pw'][:, n4 * 512:(n4 + 1) * 512]
                                  .rearrange("(k p) m -> p k m", p=P))
                for nq in range(4):
                    no = n4 * 4 + nq
                    osb = outp.tile([P, L], BF16, name="osb")
                    for tc2 in range(2):
                        ps = mmps.tile([P, 512], F32, name="pmm")
                        for kk in range(KC):
                            nc.tensor.matmul(ps[:], pwc[:, kk, nq * P:(nq + 1) * P],
                                             gT[:, kk, tc2 * 512:(tc2 + 1) * 512],
                                             start=(kk == 0), stop=(kk == KC - 1))
                        if no % 2 == 0:
                            nc.scalar.copy(osb[:, tc2 * 512:(tc2 + 1) * 512], ps[:])
                        else:
                            nc.vector.tensor_copy(osb[:, tc2 * 512:(tc2 + 1) * 512], ps[:])
                    nc.sync.dma_start(outT[no * P:(no + 1) * P, :], osb[:])

    nc.compile()
    _BUILD_CACHE[key] = nc
    return nc


LAST_RESULTS = None


def kernel(**inputs):
    global LAST_RESULTS
    from concourse import bass_utils
    sh, xts, pred_b, flags = _prepare(inputs)
    nc = _build(flags)
    in_maps = []
    for c in range(_NCORES):
        m = dict(sh)
        m['xT'] = xts[c % B]
        in_maps.append(m)
    trace = os.environ.get("BT_TRACE", "0") == "1"
    if trace:
        try:
            import antenv.axon_hooks  # noqa: F401
        except ImportError:
            trace = False
    res = bass_utils.run_bass_kernel_spmd(nc, in_maps, core_ids=list(range(_NCORES)),
                                          trace=trace)
    LAST_RESULTS = res
    outs = []
    for c in range(B):
        oT = np.asarray(res.results[c % _NCORES]['outT'], dtype=np.float32)  # [N_E, L]
        o = oT.T[_WM_INV] + pred_b[None, :]        # [L, N_E] raster order
        outs.append(o)
    return np.stack(outs).astype(np.float32)


# revision 17
# speedup vs baseline: 1.5084x; 1.2347x over previous
import os
import numpy as np

# ---- problem constants (hardcoded; kernel.py must be self-contained) ----
IMG, WS, SHIFT = 32, 8, 4
C, HEADS, DEPTH = 512, 16, 24
E_DIM, N_E, B = 256, 8192, 8
L = IMG * IMG            # 1024
NW = WS * WS             # 64 tokens per window
HD = C // HEADS          # 32
NWIN = (IMG // WS) ** 2  # 16
FH = 4 * C               # 2048
P = 128
VBLK = HD + 2            # 34 (32 vals + softmax-denominator col + pad)
VW = HEADS * VBLK        # 544
NT = L // P              # 8 token tiles
KC = C // P              # 4 k-tiles over C
KE = E_DIM // P          # 2 k-tiles over E_DIM
SCALE = HD ** -0.5
G = IMG // WS            # 4 windows per side

_DEPTH = int(os.environ.get("BT_DEPTH", DEPTH))
_NCORES = int(os.environ.get("BT_NCORES", 8))
_TP = os.environ.get("BT_TP", "pe")  # 'dma' or 'pe' transposes
_STAGE = int(os.environ.get("BT_STAGE", "99"))
_ATT = int(os.environ.get("BT_ATT", "3"))


# ---- host-side helpers (mirror reference.py) ----
def _rel_index():
    coords = np.stack(np.meshgrid(np.arange(WS), np.arange(WS), indexing='ij'))
    cf = coords.reshape(2, -1)
    rel = (cf[:, :, None] - cf[:, None, :]).transpose(1, 2, 0)
    rel[:, :, 0] += WS - 1
    rel[:, :, 1] += WS - 1
    rel[:, :, 0] *= 2 * WS - 1
    return rel.sum(-1)  # [NW, NW] int


def _shift_mask():
    img = np.zeros((IMG, IMG), np.float32)
    cnt = 0
    sl = (slice(0, -WS), slice(-WS, -SHIFT), slice(-SHIFT, None))
    for hs in sl:
        for ws_ in sl:
            img[hs, ws_] = cnt
            cnt += 1
    win = img.reshape(IMG // WS, WS, IMG // WS, WS).transpose(0, 2, 1, 3).reshape(-1, NW)
    diff = win[:, None, :] - win[:, :, None]
    return np.where(diff != 0, -100.0, 0.0).astype(np.float32)  # [NWIN, NW, NW]


def _win_perm():
    t = np.arange(L).reshape(IMG, IMG)
    wm = t.reshape(IMG // WS, WS, IMG // WS, WS).transpose(0, 2, 1, 3).reshape(-1)
    inv = np.empty(L, np.int64)
    inv[wm] = np.arange(L)
    return wm, inv  # wm: dev->raster, inv: raster->dev


_WM, _WM_INV = _win_perm()
_REL = _rel_index()
_MASK = _shift_mask()

# representative window index per pattern type for the shifted-layer mask:
#   0: interior (no mask), 1: right-column, 2: bottom-row, 3: corner
_PTYPE_WIN = [0, 3, 12, 15]
# window-pair (wp2) -> pair-type index into EE tiles (shift layers)
#   wp2 = 2a + c ; pair windows (4a+2c, 4a+2c+1)
_WP2_PT = [0 if (wp2 // 2) < 3 else 2 for wp2 in range(NT)]
for _wp2 in range(NT):
    if _wp2 % 2 == 1:
        _WP2_PT[_wp2] += 1
# per pair-type, the window type of (w01=0, w01=1)
_PT_WTYPES = [(0, 0), (0, 1), (2, 2), (2, 3)]
_WTYPE_MASK = {0: np.zeros((NW, NW), np.float32),
               1: None, 2: None, 3: None}


def _wtype_masks():
    m = {0: np.zeros((NW, NW), np.float32)}
    m[1] = _MASK[3]
    m[2] = _MASK[12]
    m[3] = _MASK[15]
    return m


def _prepare(inputs):
    import ml_dtypes
    bf16 = ml_dtypes.bfloat16
    f32 = lambda a: np.ascontiguousarray(a, dtype=np.float32)
    b16 = lambda a: np.ascontiguousarray(np.asarray(a, np.float32).astype(bf16))
    x = np.asarray(inputs['x'], np.float32)
    dec_w = np.asarray(inputs['dec_w'], np.float32)
    dec_b = np.asarray(inputs['dec_b'], np.float32)
    pos = np.asarray(inputs['pos_embed'], np.float32)[0]
    n1w = np.asarray(inputs['n1w'], np.float32)
    n1b = np.asarray(inputs['n1b'], np.float32)
    qkv_w = np.asarray(inputs['qkv_w'], np.float32)
    qkv_b = np.asarray(inputs['qkv_b'], np.float32)
    proj_w = np.asarray(inputs['proj_w'], np.float32)
    proj_b = np.asarray(inputs['proj_b'], np.float32)
    rel_bias = np.asarray(inputs['rel_bias'], np.float32)
    n2w = np.asarray(inputs['n2w'], np.float32)
    n2b = np.asarray(inputs['n2b'], np.float32)
    fc1_w = np.asarray(inputs['fc1_w'], np.float32)
    fc1_b = np.asarray(inputs['fc1_b'], np.float32)
    fc2_w = np.asarray(inputs['fc2_w'], np.float32)
    fc2_b = np.asarray(inputs['fc2_b'], np.float32)
    normf_w = np.asarray(inputs['normf_w'], np.float32)
    normf_b = np.asarray(inputs['normf_b'], np.float32)
    pred_w = np.asarray(inputs['pred_w'], np.float32)
    pred_b = np.asarray(inputs['pred_b'], np.float32)

    D = _DEPTH
    sh = {}
    sh['decw'] = b16(dec_w.T)                       # [E, C]
    sh['posb'] = b16((pos + dec_b[None, :])[_WM])   # [L, C]

    wqk = np.empty((D, C, 2 * C), bf16)
    bqk = np.zeros((D, P, 8), np.float32)
    wvp = np.zeros((D, C, VW), bf16)
    vb = np.zeros((D, P, VW), np.float32)
    ee = np.empty((D, 4, 4, P, 256), bf16)
    wp_ = np.empty((D, C, C), bf16)
    pb = np.empty((D, P, C), np.float32)
    f1 = np.empty((D, C, FH), bf16)
    f1b = np.empty((D, P, FH // P), np.float32)
    f2 = np.empty((D, FH, C), bf16)
    f2b = np.empty((D, P, C), np.float32)

    wtm = _wtype_masks()

    for i in range(D):
        Wm = qkv_w[i] * n1w[i][None, :]
        bm = qkv_w[i] @ n1b[i] + qkv_b[i]
        Wm = Wm.copy()
        bm = bm.copy()
        Wm[:C] *= SCALE
        bm[:C] *= SCALE
        wqk[i] = Wm[:2 * C].T.astype(bf16)
        bqk[i] = np.asarray(bm[:2 * C].reshape(8, P).T, np.float32)
        for h in range(HEADS):
            wvp[i][:, h * VBLK:h * VBLK + HD] = Wm[2 * C + h * HD:2 * C + (h + 1) * HD].T.astype(bf16)
            vb[i][:, h * VBLK:h * VBLK + HD] = bm[2 * C + h * HD:2 * C + (h + 1) * HD][None, :]
            vb[i][:, h * VBLK + HD] = 1.0
        # multiplicative attention bias: EE[pt, hs][p = w01*64 + tk, s*64 + tq]
        #   = exp(B_h[tq, tk] + mask[wtype][tq, tq->tk])  (transposed into [tk, tq])
        bias = rel_bias[i][_REL]                  # [tq, tk, HEADS]
        shift = (i % 2) == 1
        for pt in range(4):
            wt0, wt1 = _PT_WTYPES[pt]
            for rg in range(4):
                for hi in range(4):
                    h = hi * 4 + rg
                    a = bias[:, :, h].T           # [tk, tq]
                    for w01, wt in ((0, wt0), (1, wt1)):
                        m = wtm[wt].T if shift else wtm[0]
                        e = np.exp(a + m)         # [tk, tq]
                        ee[i, pt, rg, w01 * NW:(w01 + 1) * NW, hi * NW:(hi + 1) * NW] = e.astype(bf16)
        wp_[i] = proj_w[i].T.astype(bf16)
        pb[i] = np.broadcast_to(proj_b[i][None, :], (P, C))
        f1[i] = (fc1_w[i] * n2w[i][None, :]).T.astype(bf16)
        f1b[i] = np.asarray((fc1_w[i] @ n2b[i] + fc1_b[i]).reshape(FH // P, P).T, np.float32)
        f2[i] = fc2_w[i].T.astype(bf16)
        f2b[i] = np.broadcast_to(fc2_b[i][None, :], (P, C))

    sh['wqk'] = wqk
    sh['bqk'] = bqk
    sh['wvp'] = wvp
    sh['vb'] = vb
    sh['ee'] = np.ascontiguousarray(ee)
    sh['wp'] = wp_
    sh['pb'] = pb
    sh['f1'] = f1
    sh['f1b'] = f1b
    sh['f2'] = f2
    sh['f2b'] = f2b
    sh['nfw'] = f32(normf_w.reshape(KC, P).T)     # [P, KC]
    sh['nfb'] = f32(normf_b.reshape(KC, P).T)
    sh['pw'] = b16(pred_w.T)                      # [C, N_E]
    xts = [np.ascontiguousarray(x[c][_WM].T.astype(bf16)) for c in range(B)]
    flags = {
        'bqk0': bool(np.all(bqk == 0.0)),
        'pb0': bool(np.all(pb == 0.0)),
        'f2b0': bool(np.all(f2b == 0.0)),
    }
    return sh, xts, np.asarray(pred_b, np.float32), flags


# ---- device program ----
_BUILD_CACHE = {}


def _build(flags):
    key = (_DEPTH, _TP, _STAGE, _ATT, flags['bqk0'], flags['pb0'], flags['f2b0'])
    if key in _BUILD_CACHE:
        return _BUILD_CACHE[key]
    import concourse.bass as bass
    import concourse.mybir as mybir
    import concourse.tile as tile
    from concourse import bacc
    from concourse.masks import make_identity
    from contextlib import ExitStack

    F32 = mybir.dt.float32
    BF16 = mybir.dt.bfloat16
    AF = mybir.ActivationFunctionType
    ALU = mybir.AluOpType
    AX = mybir.AxisListType
    D = _DEPTH

    nc = bacc.Bacc("TRN2", target_bir_lowering=False, debug=False, num_devices=_NCORES)

    dr = {}
    def din(name, shape, dt):
        dr[name] = nc.dram_tensor(name, list(shape), dt, kind="ExternalInput").ap()
    din('xT', (E_DIM, L), BF16)
    din('decw', (E_DIM, C), BF16)
    din('posb', (L, C), BF16)
    din('wqk', (D, C, 2 * C), BF16)
    din('bqk', (D, P, 8), F32)
    din('wvp', (D, C, VW), BF16)
    din('vb', (D, P, VW), F32)
    din('ee', (D, 4, 4, P, 256), BF16)
    din('wp', (D, C, C), BF16)
    din('pb', (D, P, C), F32)
    din('f1', (D, C, FH), BF16)
    din('f1b', (D, P, FH // P), F32)
    din('f2', (D, FH, C), BF16)
    din('f2b', (D, P, C), F32)
    din('nfw', (P, KC), F32)
    din('nfb', (P, KC), F32)
    din('pw', (C, N_E), BF16)
    outT = nc.dram_tensor("outT", [N_E, L], BF16, kind="ExternalOutput").ap()

    with tile.TileContext(nc) as tc, ExitStack() as ES:
        cst = ES.enter_context(tc.tile_pool(name="cst", bufs=1))
        ident_b = None
        if _TP == 'pe':
            ident_f = cst.tile([P, P], F32)
            make_identity(nc, ident_f)
            ident_b = cst.tile([P, P], BF16)
            nc.scalar.copy(ident_b[:], ident_f[:])
        eps_t = cst.tile([P, 1], F32)
        nc.vector.memset(eps_t[:], 1e-5)
        dmy = cst.tile([P, 1], F32, name="dmy")

        def prefetch_act(func):
            nc.scalar.activation(dmy[:], eps_t[:], func)

        xp = ES.enter_context(tc.tile_pool(name="xp", bufs=1))
        hp = ES.enter_context(tc.tile_pool(name="hp", bufs=2))
        hTp = ES.enter_context(tc.tile_pool(name="hTp", bufs=2))
        qkp = ES.enter_context(tc.tile_pool(name="qkp", bufs=1))
        vp = ES.enter_context(tc.tile_pool(name="vp", bufs=1))
        attp = ES.enter_context(tc.tile_pool(name="attp", bufs=1))
        ptp = ES.enter_context(tc.tile_pool(name="ptp", bufs=2))
        eep = ES.enter_context(tc.tile_pool(name="eep", bufs=1))
        stp = ES.enter_context(tc.tile_pool(name="stp", bufs=2))
        recp = ES.enter_context(tc.tile_pool(name="recp", bufs=2))
        wqkp = ES.enter_context(tc.tile_pool(name="wqkp", bufs=2))
        wvpp = ES.enter_context(tc.tile_pool(name="wvpp", bufs=2))
        wpp = ES.enter_context(tc.tile_pool(name="wpp", bufs=2))
        f1p = ES.enter_context(tc.tile_pool(name="f1p", bufs=2))
        f2p = ES.enter_context(tc.tile_pool(name="f2p", bufs=2))
        gp = ES.enter_context(tc.tile_pool(name="gp", bufs=3))
        bp = ES.enter_context(tc.tile_pool(name="bp", bufs=2))
        outp = ES.enter_context(tc.tile_pool(name="outp", bufs=2))
        pwp = ES.enter_context(tc.tile_pool(name="pwp", bufs=2))

        x = xp.tile([P, NT, C], F32)

        # ---------- dec: x = xT.T @ decw + (pos + dec_b) ----------
        with tc.tile_pool(name="decp", bufs=1) as decp, \
             tc.tile_pool(name="dps", bufs=2, space="PSUM") as dps:
            xT_sb = decp.tile([P, KE, L], BF16)
            nc.sync.dma_start(xT_sb[:], dr['xT'].rearrange("(k p) t -> p k t", p=P))
            decw_sb = decp.tile([P, KE, C], BF16)
            nc.sync.dma_start(decw_sb[:], dr['decw'].rearrange("(k p) c -> p k c", p=P))
            pos_t = decp.tile([P, NT, C], BF16, name="pos_t", tag="pos")
            nc.sync.dma_start(pos_t[:], dr['posb'].rearrange("(t p) c -> p t c", p=P))
            for tt in range(NT):
                ps = dps.tile([P, C], F32)
                for kk in range(KE):
                    nc.tensor.matmul(ps[:], xT_sb[:, kk, tt * P:(tt + 1) * P],
                                     decw_sb[:, kk, :], start=(kk == 0), stop=(kk == KE - 1))
                nc.vector.tensor_add(x[:, tt], ps[:], pos_t[:, tt])

        # ---------- layer-norm: h = (x - mean) * rstd  (bf16 out) ----------
        def layernorm(dst, src, pfx):
            # two tc2-aligned halves so the first half's LN output doesn't wait
            # on stats of tiles still being produced (keeps PE fed at layer edges)
            stats = stp.tile([P, NT, 6], F32, name=f"{pfx}st", tag=f"{pfx}st")
            mv = stp.tile([P, NT, 2], F32, name=f"{pfx}mv", tag=f"{pfx}mv")
            lnv = stp.tile([P, NT], F32, name=f"{pfx}ln", tag=f"{pfx}ln")
            rstd = stp.tile([P, NT], F32, name=f"{pfx}rs", tag=f"{pfx}rs")
            nb = stp.tile([P, NT], F32, name=f"{pfx}nb", tag=f"{pfx}nb")
            for hf_ in range(2):
                ts_ = slice(hf_ * 4, (hf_ + 1) * 4)
                for tt in range(hf_ * 4, (hf_ + 1) * 4):
                    nc.vector.bn_stats(stats[:, tt], src[:, tt])
                    nc.vector.bn_aggr(mv[:, tt], stats[:, tt])
                # rstd = 1/sqrt(var+eps) (sqrt on ACT, reciprocal on DVE)
                nc.scalar.activation(lnv[:, ts_], mv[:, ts_, 1], AF.Sqrt, bias=eps_t[:])
                nc.vector.reciprocal(rstd[:, ts_], lnv[:, ts_])
                nc.vector.tensor_mul(nb[:, ts_], mv[:, ts_, 0], rstd[:, ts_])
                nc.vector.tensor_scalar_mul(nb[:, ts_], nb[:, ts_], -1.0)
                for tt in range(hf_ * 4, (hf_ + 1) * 4):
                    nc.scalar.activation(dst[:, tt], src[:, tt], AF.Identity,
                                         bias=nb[:, tt:tt + 1], scale=rstd[:, tt:tt + 1])

        # transpose token-major [P, NT, C] bf16 -> C-major [P, KC, L] bf16
        def transpose_to(hT, src, tpool=None):
            if _TP == 'dma':
                for ct in range(KC):
                    for tt in range(NT):
                        nc.sync.dma_start_transpose(
                            hT[:, ct, tt * P:(tt + 1) * P],
                            src[:, tt, ct * P:(ct + 1) * P])
            else:
                for ct in range(KC):
                    for g4 in range(2):
                        tps = tpool.tile([P, 4, P], BF16, name="tp")
                        for q in range(4):
                            tt = g4 * 4 + q
                            nc.tensor.transpose(tps[:, q], src[:, tt, ct * P:(ct + 1) * P],
                                                ident_b[:])
                        nc.scalar.copy(hT[:, ct, g4 * 512:(g4 + 1) * 512],
                                       tps[:].rearrange("p a b -> p (a b)"))

        # shift permute in hT space (window-major tokens), DVE+GpSimd copies.
        # fwd: dstT(B)[RB] = srcT(A)[RA]; else dstT(A)[RA] = srcT(B)[RB]
        # ct-outer so each k-slice completes early for downstream matmuls.
        def permute(dstT, srcT, fwd):
            sv = srcT[:].rearrange("p k (a b i j) -> p k a b i j", a=G, b=G, i=WS, j=WS)
            dv = dstT[:].rearrange("p k (a b i j) -> p k a b i j", a=G, b=G, i=WS, j=WS)
            for ct in range(KC):
                n = 0
                for qa in range(2):
                    di = slice(0, 4) if qa == 0 else slice(4, 8)
                    si = slice(4, 8) if qa == 0 else slice(0, 4)
                    for qb in range(2):
                        dj = slice(0, 4) if qb == 0 else slice(4, 8)
                        sj = slice(4, 8) if qb == 0 else slice(0, 4)
                        if qb == 0:
                            bpairs = [(slice(0, G), slice(0, G))]
                        else:
                            bpairs = [(slice(0, G - 1), slice(1, G)),
                                      (slice(G - 1, G), slice(0, 1))]
                        for a in range(G):
                            sa = (a + qa) % G
                            for db, sb_ in bpairs:
                                eng = (nc.gpsimd, nc.vector)[n % 2]
                                n += 1
                                if fwd:
                                    eng.tensor_copy(dv[:, ct, a, db, di, dj],
                                                    sv[:, ct, sa, sb_, si, sj])
                                else:
                                    eng.tensor_copy(dv[:, ct, sa, sb_, si, sj],
                                                    sv[:, ct, a, db, di, dj])

        qkT = qkp.tile([P, 8, L], BF16)
        v_aug = vp.tile([P, NT, VW], BF16)
        att = attp.tile([P, NT, C], BF16)

        # ---------- layers ----------
        for i in range(D):
            shift = (i % 2) == 1
            if _STAGE < 1:
                continue
            # LN1 + transpose (+ shift permute)
            h = hp.tile([P, NT, C], BF16, name="h")
            layernorm(h, x, "l1")
            prefetch_act(AF.Exp)
            if _STAGE < 2:
                continue
            with tc.tile_pool(name="tp1", bufs=2, space="PSUM") as tpool:
                hT_A = hTp.tile([P, KC, L], BF16, name="hT")
                transpose_to(hT_A, h, tpool)
            if shift:
                hT = hTp.tile([P, KC, L], BF16, name="hT")
                permute(hT, hT_A, True)
            else:
                hT = hT_A
            if _STAGE < 3:
                continue

            # ---- qk + v ----
            wqk_sb = wqkp.tile([P, KC, 2 * C], BF16, name="wqk")
            nc.sync.dma_start(wqk_sb[:], dr['wqk'][i].rearrange("(k p) m -> p k m", p=P))
            wvp_sb = wvpp.tile([P, KC, VW], BF16, name="wvp")
            nc.sync.dma_start(wvp_sb[:], dr['wvp'][i].rearrange("(k p) m -> p k m", p=P))
            vb_sb = bp.tile([P, VW], F32, name="vb", tag="vb")
            nc.sync.dma_start(vb_sb[:], dr['vb'][i])
            bqk_sb = None
            if not flags['bqk0']:
                bqk_sb = bp.tile([P, 8], F32, name="bqk", tag="bqk")
                nc.sync.dma_start(bqk_sb[:], dr['bqk'][i])
            with tc.tile_pool(name="qkps", bufs=3, space="PSUM") as qkps, \
                 tc.tile_pool(name="vps", bufs=2, space="PSUM") as vps:
                for tc2 in range(2):
                    for mo in range(8):
                        ps = qkps.tile([P, C], F32, name="qkmm")
                        for kk in range(KC):
                            nc.tensor.matmul(ps[:], wqk_sb[:, kk, mo * P:(mo + 1) * P],
                                             hT[:, kk, tc2 * 512:(tc2 + 1) * 512],
                                             start=(kk == 0), stop=(kk == KC - 1))
                        if flags['bqk0']:
                            nc.scalar.copy(qkT[:, mo, tc2 * 512:(tc2 + 1) * 512], ps[:])
                        else:
                            nc.scalar.activation(qkT[:, mo, tc2 * 512:(tc2 + 1) * 512],
                                                 ps[:], AF.Identity,
                                                 bias=bqk_sb[:, mo:mo + 1])
                for tt in range(NT):
                    ps = vps.tile([P, VW], F32, name="vmm")
                    for kk in range(KC):
                        nc.tensor.matmul(ps[:, 0:512], hT[:, kk, tt * P:(tt + 1) * P],
                                         wvp_sb[:, kk, 0:512], start=(kk == 0),
                                         stop=(kk == KC - 1), skip_group_check=True)
                        nc.tensor.matmul(ps[:, 512:VW], hT[:, kk, tt * P:(tt + 1) * P],
                                         wvp_sb[:, kk, 512:VW], start=(kk == 0),
                                         stop=(kk == KC - 1), skip_group_check=True)
                    nc.vector.tensor_add(v_aug[:, tt], ps[:], vb_sb[:])

            if _STAGE < 4:
                continue
            # ---- attention ----
            # EE tiles for this layer
            pts_needed = sorted(set(_WP2_PT)) if shift else [0]
            ee_sb = {}
            for pt in pts_needed:
                t = eep.tile([P, 4, 256], BF16, name=f"ee{pt}", tag=f"ee{pt}")
                nc.sync.dma_start(t[:], dr['ee'][i, pt].rearrange("r p c -> p r c"))
                for rg in range(4):
                    ee_sb[(pt, rg)] = t[:, rg]
            with tc.tile_pool(name="sps", bufs=1, space="PSUM") as sps, \
                 tc.tile_pool(name="avps", bufs=2, space="PSUM") as avps:
                for wp2 in range(NT):
                    pt = _WP2_PT[wp2] if shift else 0
                    # S: one full PSUM bank per rg (row-group); MMs from different
                    # row-groups must not share a bank. rg-interleaved emission so
                    # LDWs pull ahead across row groups.
                    sgs = [sps.tile([P, 8, NW], F32, name=f"s{rg}", tag=f"s{rg}")
                           for rg in range(4)]
                    for j in range(8):
                        hi = j % 4
                        w01 = j // 4
                        wc = slice((wp2 * 2 + w01) * NW, (wp2 * 2 + w01 + 1) * NW)
                        for rg in range(4):
                            nc.tensor.matmul(
                                sgs[rg][w01 * NW:(w01 + 1) * NW, hi, :],
                                qkT[rg * HD:(rg + 1) * HD, 4 + hi, wc],
                                qkT[rg * HD:(rg + 1) * HD, hi, wc],
                                start=True, stop=True, skip_group_check=True,
                                tile_position=(rg * HD, w01 * NW))
                    if _ATT < 1:
                        continue
                    ptg = []
                    for rg in range(4):
                        er = ptp.tile([P, 4, NW], BF16, name=f"er{rg}", tag=f"er{rg}")
                        nc.scalar.activation(er[:].rearrange("p a b -> p (a b)"),
                                             sgs[rg][:, 0:4, :].rearrange("p a b -> p (a b)"),
                                             AF.Exp)
                        pt_t = ptp.tile([P, 4, NW], BF16, name=f"pt{rg}", tag=f"pt{rg}")
                        nc.vector.tensor_mul(pt_t[:].rearrange("p a b -> p (a b)"),
                                             er[:].rearrange("p a b -> p (a b)"),
                                             ee_sb[(pt, rg)])
                        ptg.append(pt_t)
                    if _ATT < 2:
                        continue
                    avs = [avps.tile([P, 8, NW], F32, name=f"av{hs}", tag=f"av{hs}")
                           for hs in range(2)]
                    for s in range(8):
                        hi0 = (s // 4) % 2
                        rg = s % 4
                        for hs in range(2):
                            hi = 2 * hs + hi0
                            h_ = hi * 4 + rg
                            for w01 in range(2):
                                rows = slice(w01 * NW, (w01 + 1) * NW)
                                nc.tensor.matmul(
                                    avs[hs][rows, s, 0:VBLK], ptg[rg][rows, hi, :],
                                    v_aug[rows, wp2, h_ * VBLK:(h_ + 1) * VBLK],
                                    start=True, stop=True, skip_group_check=True,
                                    tile_position=(w01 * NW, w01 * NW))
                    if _ATT < 3:
                        continue
                    for hs in range(2):
                        for w01 in range(2):
                            rows = slice(w01 * NW, (w01 + 1) * NW)
                            rec = recp.tile([P, 2, 8], F32, name="rec", tag=f"rec{hs}{w01}")
                            nc.vector.reciprocal(rec[rows, w01], avs[hs][rows, :, HD])
                            rb = rec[rows, w01].rearrange("p (a b) -> p a b", b=1) \
                                .to_broadcast((NW, 8, HD))
                            dst = att[rows, wp2, hs * 256:(hs + 1) * 256] \
                                .rearrange("p (a b) -> p a b", b=HD)
                            nc.vector.tensor_mul(dst, avs[hs][rows, :, 0:HD], rb)

            if _STAGE < 5:
                continue
            prefetch_act(AF.Sqrt)
            # ---- attn transpose back (+ inverse shift permute) + proj ----
            with tc.tile_pool(name="tp2", bufs=2, space="PSUM") as tpool:
                aT_B = hTp.tile([P, KC, L], BF16, name="hT")
                transpose_to(aT_B, att, tpool)
            if shift:
                aT = hTp.tile([P, KC, L], BF16, name="hT")
                permute(aT, aT_B, False)
            else:
                aT = aT_B
            wp_sb = wpp.tile([P, KC, C], BF16, name="wp")
            nc.sync.dma_start(wp_sb[:], dr['wp'][i].rearrange("(k p) m -> p k m", p=P))
            if not flags['pb0']:
                pb_sb = bp.tile([P, C], F32, name="pb", tag="pb")
                nc.sync.dma_start(pb_sb[:], dr['pb'][i])
            with tc.tile_pool(name="pjps", bufs=3, space="PSUM") as pjps:
                for tt in range(NT):
                    ps = pjps.tile([P, C], F32, name="pjmm")
                    for kk in range(KC):
                        nc.tensor.matmul(ps[:], aT[:, kk, tt * P:(tt + 1) * P],
                                         wp_sb[:, kk, :], start=(kk == 0), stop=(kk == KC - 1))
                    nc.vector.tensor_add(x[:, tt], ps[:], x[:, tt])
                    if not flags['pb0']:
                        nc.gpsimd.tensor_add(x[:, tt], x[:, tt], pb_sb[:])

            if _STAGE < 6:
                continue
            # ---- LN2 + transpose + MLP ----
            h2 = hp.tile([P, NT, C], BF16, name="h")
            layernorm(h2, x, "l2")
            prefetch_act(AF.Gelu)
            with tc.tile_pool(name="tp3", bufs=2, space="PSUM") as tpool:
                h2T = hTp.tile([P, KC, L], BF16, name="hT")
                transpose_to(h2T, h2, tpool)
            if _STAGE < 7:
                continue
            f1b_sb = bp.tile([P, FH // P], F32, name="f1b", tag="f1b")
            nc.sync.dma_start(f1b_sb[:], dr['f1b'][i])
            if not flags['f2b0']:
                f2b_sb = bp.tile([P, C], F32, name="f2b", tag="f2b")
                nc.sync.dma_start(f2b_sb[:], dr['f2b'][i])
            with tc.tile_pool(name="f1ps", bufs=3, space="PSUM") as f1ps, \
                 tc.tile_pool(name="fc2ps", bufs=1, space="PSUM") as fc2ps:
                for tc2 in range(2):
                    pso = [fc2ps.tile([P, C], F32, name=f"fc2_{j}", tag=f"fc2_{j}")
                           for j in range(4)]
                    for q in range(4):
                        f1c = f1p.tile([P, KC, 4 * P], BF16, name="f1c")
                        nc.sync.dma_start(f1c[:], dr['f1'][i][:, q * 512:(q + 1) * 512]
                                          .rearrange("(k p) m -> p k m", p=P))
                        f2c = f2p.tile([P, 4, C], BF16, name="f2c")
                        nc.sync.dma_start(f2c[:], dr['f2'][i][q * 512:(q + 1) * 512, :]
                                          .rearrange("(o p) c -> p o c", p=P))
                        for oq in range(4):
                            ho = q * 4 + oq
                            ps1 = f1ps.tile([P, C], F32, name="f1mm")
                            for kk in range(KC):
                                nc.tensor.matmul(ps1[:], f1c[:, kk, oq * P:(oq + 1) * P],
                                                 h2T[:, kk, tc2 * 512:(tc2 + 1) * 512],
                                                 start=(kk == 0), stop=(kk == KC - 1))
                            g = gp.tile([P, C], BF16, name="g")
                            nc.scalar.activation(g[:], ps1[:], AF.Gelu, bias=f1b_sb[:, ho:ho + 1])
                            for j in range(4):
                                nc.tensor.matmul(pso[j][:], g[:, j * P:(j + 1) * P],
                                                 f2c[:, oq, :],
                                                 start=(ho == 0), stop=(ho == FH // P - 1))
                    if tc2 == 1:
                        prefetch_act(AF.Sqrt)
                    for j in range(4):
                        tt = tc2 * 4 + j
                        nc.vector.tensor_add(x[:, tt], pso[j][:], x[:, tt])
                        if not flags['f2b0']:
                            nc.gpsimd.tensor_add(x[:, tt], x[:, tt], f2b_sb[:])

        # ---------- final LN + gelu + pred ----------
        hf = hp.tile([P, NT, C], BF16, name="h")
        layernorm(hf, x, "lf")
        with tc.tile_pool(name="tpf", bufs=2, space="PSUM") as tpool:
            hfT = hTp.tile([P, KC, L], BF16, name="hT")
            transpose_to(hfT, hf, tpool)
        nfw_sb = bp.tile([P, KC], F32, name="nfw", tag="nfw")
        nc.sync.dma_start(nfw_sb[:], dr['nfw'])
        nfb_sb = bp.tile([P, KC], F32, name="nfb", tag="nfb")
        nc.sync.dma_start(nfb_sb[:], dr['nfb'])
        gT = hTp.tile([P, KC, L], BF16, name="gT", bufs=1)
        for ct in range(KC):
            nc.scalar.activation(gT[:, ct], hfT[:, ct], AF.Gelu,
                                 bias=nfb_sb[:, ct:ct + 1], scale=nfw_sb[:, ct:ct + 1])
        with tc.tile_pool(name="mmpsf", bufs=4, space="PSUM") as mmps:
            for n4 in range(N_E // (4 * P)):
                pwc = pwp.tile([P, KC, 4 * P], BF16, name="pwc")
                nc.sync.dma_start(pwc[:], dr['pw'][:, n4 * 512:(n4 + 1) * 512]
                                  .rearrange("(k p) m -> p k m", p=P))
                for nq in range(4):
                    no = n4 * 4 + nq
                    osb = outp.tile([P, L], BF16, name="osb")
                    for tc2 in range(2):
                        ps = mmps.tile([P, 512], F32, name="pmm")
                        for kk in range(KC):
                            nc.tensor.matmul(ps[:], pwc[:, kk, nq * P:(nq + 1) * P],
                                             gT[:, kk, tc2 * 512:(tc2 + 1) * 512],
                                             start=(kk == 0), stop=(kk == KC - 1))
                        if no % 2 == 0:
                            nc.scalar.copy(osb[:, tc2 * 512:(tc2 + 1) * 512], ps[:])
                        else:
                            nc.vector.tensor_copy(osb[:, tc2 * 512:(tc2 + 1) * 512], ps[:])
                    nc.sync.dma_start(outT[no * P:(no + 1) * P, :], osb[:])

    nc.compile()
    _BUILD_CACHE[key] = nc
    return nc


LAST_RESULTS = None


def kernel(**inputs):
    global LAST_RESULTS
    from concourse import bass_utils
    sh, xts, pred_b, flags = _prepare(inputs)
    nc = _build(flags)
    in_maps = []
    for c in range(_NCORES):
        m = dict(sh)
        m['xT'] = xts[c % B]
        in_maps.append(m)
    trace = os.environ.get("BT_TRACE", "0") == "1"
    if trace:
        try:
            import antenv.axon_hooks  # noqa: F401
        except ImportError:
            trace = False
    res = bass_utils.run_bass_kernel_spmd(nc, in_maps, core_ids=list(range(_NCORES)),
                                          trace=trace)
    LAST_RESULTS = res
    outs = []
    for c in range(B):
        oT = np.asarray(res.results[c % _NCORES]['outT'], dtype=np.float32)  # [N_E, L]
        o = oT.T[_WM_INV] + pred_b[None, :]        # [L, N_E] raster order
        outs.append(o)
    return np.stack(outs).astype(np.float32)


# revision 18
# speedup vs baseline: 1.6236x; 1.0763x over previous
import os
import numpy as np

# ---- problem constants (hardcoded; kernel.py must be self-contained) ----
IMG, WS, SHIFT = 32, 8, 4
C, HEADS, DEPTH = 512, 16, 24
E_DIM, N_E, B = 256, 8192, 8
L = IMG * IMG            # 1024
NW = WS * WS             # 64 tokens per window
HD = C // HEADS          # 32
NWIN = (IMG // WS) ** 2  # 16
FH = 4 * C               # 2048
P = 128
VBLK = HD + 2            # 34 (32 vals + softmax-denominator col + pad)
VW = HEADS * VBLK        # 544
NT = L // P              # 8 token tiles
KC = C // P              # 4 k-tiles over C
KE = E_DIM // P          # 2 k-tiles over E_DIM
SCALE = HD ** -0.5
G = IMG // WS            # 4 windows per side

_DEPTH = int(os.environ.get("BT_DEPTH", DEPTH))
_NCORES = int(os.environ.get("BT_NCORES", 8))
_TP = os.environ.get("BT_TP", "pe")  # 'dma' or 'pe' transposes
_STAGE = int(os.environ.get("BT_STAGE", "99"))
_ATT = int(os.environ.get("BT_ATT", "3"))


# ---- host-side helpers (mirror reference.py) ----
def _rel_index():
    coords = np.stack(np.meshgrid(np.arange(WS), np.arange(WS), indexing='ij'))
    cf = coords.reshape(2, -1)
    rel = (cf[:, :, None] - cf[:, None, :]).transpose(1, 2, 0)
    rel[:, :, 0] += WS - 1
    rel[:, :, 1] += WS - 1
    rel[:, :, 0] *= 2 * WS - 1
    return rel.sum(-1)  # [NW, NW] int


def _shift_mask():
    img = np.zeros((IMG, IMG), np.float32)
    cnt = 0
    sl = (slice(0, -WS), slice(-WS, -SHIFT), slice(-SHIFT, None))
    for hs in sl:
        for ws_ in sl:
            img[hs, ws_] = cnt
            cnt += 1
    win = img.reshape(IMG // WS, WS, IMG // WS, WS).transpose(0, 2, 1, 3).reshape(-1, NW)
    diff = win[:, None, :] - win[:, :, None]
    return np.where(diff != 0, -100.0, 0.0).astype(np.float32)  # [NWIN, NW, NW]


def _win_perm():
    t = np.arange(L).reshape(IMG, IMG)
    wm = t.reshape(IMG // WS, WS, IMG // WS, WS).transpose(0, 2, 1, 3).reshape(-1)
    inv = np.empty(L, np.int64)
    inv[wm] = np.arange(L)
    return wm, inv  # wm: dev->raster, inv: raster->dev


_WM, _WM_INV = _win_perm()
_REL = _rel_index()
_MASK = _shift_mask()

# representative window index per pattern type for the shifted-layer mask:
#   0: interior (no mask), 1: right-column, 2: bottom-row, 3: corner
_PTYPE_WIN = [0, 3, 12, 15]
# window-pair (wp2) -> pair-type index into EE tiles (shift layers)
#   wp2 = 2a + c ; pair windows (4a+2c, 4a+2c+1)
_WP2_PT = [0 if (wp2 // 2) < 3 else 2 for wp2 in range(NT)]
for _wp2 in range(NT):
    if _wp2 % 2 == 1:
        _WP2_PT[_wp2] += 1
# per pair-type, the window type of (w01=0, w01=1)
_PT_WTYPES = [(0, 0), (0, 1), (2, 2), (2, 3)]
_WTYPE_MASK = {0: np.zeros((NW, NW), np.float32),
               1: None, 2: None, 3: None}


def _wtype_masks():
    m = {0: np.zeros((NW, NW), np.float32)}
    m[1] = _MASK[3]
    m[2] = _MASK[12]
    m[3] = _MASK[15]
    return m


def _prepare(inputs):
    import ml_dtypes
    bf16 = ml_dtypes.bfloat16
    f32 = lambda a: np.ascontiguousarray(a, dtype=np.float32)
    b16 = lambda a: np.ascontiguousarray(np.asarray(a, np.float32).astype(bf16))
    x = np.asarray(inputs['x'], np.float32)
    dec_w = np.asarray(inputs['dec_w'], np.float32)
    dec_b = np.asarray(inputs['dec_b'], np.float32)
    pos = np.asarray(inputs['pos_embed'], np.float32)[0]
    n1w = np.asarray(inputs['n1w'], np.float32)
    n1b = np.asarray(inputs['n1b'], np.float32)
    qkv_w = np.asarray(inputs['qkv_w'], np.float32)
    qkv_b = np.asarray(inputs['qkv_b'], np.float32)
    proj_w = np.asarray(inputs['proj_w'], np.float32)
    proj_b = np.asarray(inputs['proj_b'], np.float32)
    rel_bias = np.asarray(inputs['rel_bias'], np.float32)
    n2w = np.asarray(inputs['n2w'], np.float32)
    n2b = np.asarray(inputs['n2b'], np.float32)
    fc1_w = np.asarray(inputs['fc1_w'], np.float32)
    fc1_b = np.asarray(inputs['fc1_b'], np.float32)
    fc2_w = np.asarray(inputs['fc2_w'], np.float32)
    fc2_b = np.asarray(inputs['fc2_b'], np.float32)
    normf_w = np.asarray(inputs['normf_w'], np.float32)
    normf_b = np.asarray(inputs['normf_b'], np.float32)
    pred_w = np.asarray(inputs['pred_w'], np.float32)
    pred_b = np.asarray(inputs['pred_b'], np.float32)

    D = _DEPTH
    sh = {}
    sh['decw'] = b16(dec_w.T)                       # [E, C]
    sh['posb'] = b16((pos + dec_b[None, :])[_WM])   # [L, C]

    wqk = np.empty((D, C, 2 * C), bf16)
    bqk = np.zeros((D, P, 8), np.float32)
    wvp = np.zeros((D, C, VW), bf16)
    vb = np.zeros((D, P, VW), np.float32)
    ee = np.empty((D, 4, 4, P, 256), bf16)
    wp_ = np.empty((D, C, C), bf16)
    pb = np.empty((D, P, C), np.float32)
    f1 = np.empty((D, C, FH), bf16)
    f1b = np.empty((D, P, FH // P), np.float32)
    f2 = np.empty((D, FH, C), bf16)
    f2b = np.empty((D, P, C), np.float32)

    wtm = _wtype_masks()

    for i in range(D):
        Wm = qkv_w[i] * n1w[i][None, :]
        bm = qkv_w[i] @ n1b[i] + qkv_b[i]
        Wm = Wm.copy()
        bm = bm.copy()
        Wm[:C] *= SCALE
        bm[:C] *= SCALE
        wqk[i] = Wm[:2 * C].T.astype(bf16)
        bqk[i] = np.asarray(bm[:2 * C].reshape(8, P).T, np.float32)
        for h in range(HEADS):
            wvp[i][:, h * VBLK:h * VBLK + HD] = Wm[2 * C + h * HD:2 * C + (h + 1) * HD].T.astype(bf16)
            vb[i][:, h * VBLK:h * VBLK + HD] = bm[2 * C + h * HD:2 * C + (h + 1) * HD][None, :]
            vb[i][:, h * VBLK + HD] = 1.0
        # multiplicative attention bias: EE[pt, hs][p = w01*64 + tk, s*64 + tq]
        #   = exp(B_h[tq, tk] + mask[wtype][tq, tq->tk])  (transposed into [tk, tq])
        bias = rel_bias[i][_REL]                  # [tq, tk, HEADS]
        shift = (i % 2) == 1
        for pt in range(4):
            wt0, wt1 = _PT_WTYPES[pt]
            for rg in range(4):
                for hi in range(4):
                    h = hi * 4 + rg
                    a = bias[:, :, h].T           # [tk, tq]
                    for w01, wt in ((0, wt0), (1, wt1)):
                        m = wtm[wt].T if shift else wtm[0]
                        e = np.exp(a + m)         # [tk, tq]
                        ee[i, pt, rg, w01 * NW:(w01 + 1) * NW, hi * NW:(hi + 1) * NW] = e.astype(bf16)
        wp_[i] = proj_w[i].T.astype(bf16)
        pb[i] = np.broadcast_to(proj_b[i][None, :], (P, C))
        f1[i] = (fc1_w[i] * n2w[i][None, :]).T.astype(bf16)
        f1b[i] = np.asarray((fc1_w[i] @ n2b[i] + fc1_b[i]).reshape(FH // P, P).T, np.float32)
        f2[i] = fc2_w[i].T.astype(bf16)
        f2b[i] = np.broadcast_to(fc2_b[i][None, :], (P, C))

    sh['wqk'] = wqk
    sh['bqk'] = bqk
    sh['wvp'] = wvp
    sh['vb'] = vb
    sh['ee'] = np.ascontiguousarray(ee)
    sh['wp'] = wp_
    sh['pb'] = pb
    sh['f1'] = f1
    sh['f1b'] = f1b
    sh['f2'] = f2
    sh['f2b'] = f2b
    sh['nfw'] = f32(normf_w.reshape(KC, P).T)     # [P, KC]
    sh['nfb'] = f32(normf_b.reshape(KC, P).T)
    sh['pw'] = b16(pred_w.T)                      # [C, N_E]
    xts = [np.ascontiguousarray(x[c][_WM].T.astype(bf16)) for c in range(B)]
    flags = {
        'bqk0': bool(np.all(bqk == 0.0)),
        'pb0': bool(np.all(pb == 0.0)),
        'f2b0': bool(np.all(f2b == 0.0)),
    }
    return sh, xts, np.asarray(pred_b, np.float32), flags


# ---- device program ----
_BUILD_CACHE = {}


def _build(flags):
    key = (_DEPTH, _TP, _STAGE, _ATT, flags['bqk0'], flags['pb0'], flags['f2b0'])
    if key in _BUILD_CACHE:
        return _BUILD_CACHE[key]
    import concourse.bass as bass
    import concourse.mybir as mybir
    import concourse.tile as tile
    from concourse import bacc
    from concourse.masks import make_identity
    from contextlib import ExitStack

    F32 = mybir.dt.float32
    BF16 = mybir.dt.bfloat16
    AF = mybir.ActivationFunctionType
    ALU = mybir.AluOpType
    AX = mybir.AxisListType
    D = _DEPTH

    nc = bacc.Bacc("TRN2", target_bir_lowering=False, debug=False, num_devices=_NCORES)

    dr = {}
    def din(name, shape, dt):
        dr[name] = nc.dram_tensor(name, list(shape), dt, kind="ExternalInput").ap()
    din('xT', (E_DIM, L), BF16)
    din('decw', (E_DIM, C), BF16)
    din('posb', (L, C), BF16)
    din('wqk', (D, C, 2 * C), BF16)
    din('bqk', (D, P, 8), F32)
    din('wvp', (D, C, VW), BF16)
    din('vb', (D, P, VW), F32)
    din('ee', (D, 4, 4, P, 256), BF16)
    din('wp', (D, C, C), BF16)
    din('pb', (D, P, C), F32)
    din('f1', (D, C, FH), BF16)
    din('f1b', (D, P, FH // P), F32)
    din('f2', (D, FH, C), BF16)
    din('f2b', (D, P, C), F32)
    din('nfw', (P, KC), F32)
    din('nfb', (P, KC), F32)
    din('pw', (C, N_E), BF16)
    outT = nc.dram_tensor("outT", [N_E, L], BF16, kind="ExternalOutput").ap()

    with tile.TileContext(nc) as tc, ExitStack() as ES:
        cst = ES.enter_context(tc.tile_pool(name="cst", bufs=1))
        ident_b = None
        if _TP == 'pe':
            ident_f = cst.tile([P, P], F32)
            make_identity(nc, ident_f)
            ident_b = cst.tile([P, P], BF16)
            nc.scalar.copy(ident_b[:], ident_f[:])
        eps_t = cst.tile([P, 1], F32)
        nc.vector.memset(eps_t[:], 1e-5)
        dmy = cst.tile([P, 1], F32, name="dmy")

        def prefetch_act(func):
            nc.scalar.activation(dmy[:], eps_t[:], func)

        xp = ES.enter_context(tc.tile_pool(name="xp", bufs=1))
        hp = ES.enter_context(tc.tile_pool(name="hp", bufs=2))
        hTp = ES.enter_context(tc.tile_pool(name="hTp", bufs=2))
        qkp = ES.enter_context(tc.tile_pool(name="qkp", bufs=1))
        vp = ES.enter_context(tc.tile_pool(name="vp", bufs=1))
        attp = ES.enter_context(tc.tile_pool(name="attp", bufs=1))
        ptp = ES.enter_context(tc.tile_pool(name="ptp", bufs=2))
        eep = ES.enter_context(tc.tile_pool(name="eep", bufs=1))
        stp = ES.enter_context(tc.tile_pool(name="stp", bufs=2))
        recp = ES.enter_context(tc.tile_pool(name="recp", bufs=2))
        wqkp = ES.enter_context(tc.tile_pool(name="wqkp", bufs=2))
        wvpp = ES.enter_context(tc.tile_pool(name="wvpp", bufs=2))
        wpp = ES.enter_context(tc.tile_pool(name="wpp", bufs=2))
        f1p = ES.enter_context(tc.tile_pool(name="f1p", bufs=2))
        f2p = ES.enter_context(tc.tile_pool(name="f2p", bufs=2))
        gp = ES.enter_context(tc.tile_pool(name="gp", bufs=3))
        bp = ES.enter_context(tc.tile_pool(name="bp", bufs=2))
        outp = ES.enter_context(tc.tile_pool(name="outp", bufs=2))
        pwp = ES.enter_context(tc.tile_pool(name="pwp", bufs=2))

        x = xp.tile([P, NT, C], F32)

        # ---------- dec: x = xT.T @ decw + (pos + dec_b) ----------
        with tc.tile_pool(name="decp", bufs=1) as decp, \
             tc.tile_pool(name="dps", bufs=2, space="PSUM") as dps:
            xT_sb = decp.tile([P, KE, L], BF16)
            nc.sync.dma_start(xT_sb[:], dr['xT'].rearrange("(k p) t -> p k t", p=P))
            decw_sb = decp.tile([P, KE, C], BF16)
            nc.sync.dma_start(decw_sb[:], dr['decw'].rearrange("(k p) c -> p k c", p=P))
            pos_t = decp.tile([P, NT, C], BF16, name="pos_t", tag="pos")
            nc.sync.dma_start(pos_t[:], dr['posb'].rearrange("(t p) c -> p t c", p=P))
            for tt in range(NT):
                ps = dps.tile([P, C], F32)
                for kk in range(KE):
                    nc.tensor.matmul(ps[:], xT_sb[:, kk, tt * P:(tt + 1) * P],
                                     decw_sb[:, kk, :], start=(kk == 0), stop=(kk == KE - 1))
                nc.vector.tensor_add(x[:, tt], ps[:], pos_t[:, tt])

        # ---------- layer-norm: h = (x - mean) * rstd  (bf16 out) ----------
        def layernorm(dst, src, pfx):
            # two tc2-aligned halves so the first half's LN output doesn't wait
            # on stats of tiles still being produced (keeps PE fed at layer edges)
            stats = stp.tile([P, NT, 6], F32, name=f"{pfx}st", tag=f"{pfx}st")
            mv = stp.tile([P, NT, 2], F32, name=f"{pfx}mv", tag=f"{pfx}mv")
            lnv = stp.tile([P, NT], F32, name=f"{pfx}ln", tag=f"{pfx}ln")
            rstd = stp.tile([P, NT], F32, name=f"{pfx}rs", tag=f"{pfx}rs")
            nb = stp.tile([P, NT], F32, name=f"{pfx}nb", tag=f"{pfx}nb")
            for hf_ in range(2):
                ts_ = slice(hf_ * 4, (hf_ + 1) * 4)
                for tt in range(hf_ * 4, (hf_ + 1) * 4):
                    nc.vector.bn_stats(stats[:, tt], src[:, tt])
                    nc.vector.bn_aggr(mv[:, tt], stats[:, tt])
                # rstd = 1/sqrt(var+eps) (sqrt on ACT, reciprocal on DVE)
                nc.scalar.activation(lnv[:, ts_], mv[:, ts_, 1], AF.Sqrt, bias=eps_t[:])
                nc.vector.reciprocal(rstd[:, ts_], lnv[:, ts_])
                nc.vector.tensor_mul(nb[:, ts_], mv[:, ts_, 0], rstd[:, ts_])
                nc.vector.tensor_scalar_mul(nb[:, ts_], nb[:, ts_], -1.0)
                for tt in range(hf_ * 4, (hf_ + 1) * 4):
                    nc.gpsimd.tensor_scalar(dst[:, tt], src[:, tt],
                                            rstd[:, tt:tt + 1], nb[:, tt:tt + 1],
                                            ALU.mult, ALU.add)

        # transpose token-major [P, NT, C] bf16 -> C-major [P, KC, L] bf16
        def transpose_to(hT, src, tpool=None):
            if _TP == 'dma':
                for ct in range(KC):
                    for tt in range(NT):
                        nc.sync.dma_start_transpose(
                            hT[:, ct, tt * P:(tt + 1) * P],
                            src[:, tt, ct * P:(ct + 1) * P])
            else:
                for g4 in range(2):
                    for ct in range(KC):
                        tps = tpool.tile([P, 4, P], BF16, name="tp")
                        for q in range(4):
                            tt = g4 * 4 + q
                            nc.tensor.transpose(tps[:, q], src[:, tt, ct * P:(ct + 1) * P],
                                                ident_b[:])
                        nc.scalar.copy(hT[:, ct, g4 * 512:(g4 + 1) * 512],
                                       tps[:].rearrange("p a b -> p (a b)"))

        # shift permute in hT space (window-major tokens), DVE+GpSimd copies.
        # fwd: dstT(B)[RB] = srcT(A)[RA]; else dstT(A)[RA] = srcT(B)[RB]
        # ct-outer so each k-slice completes early for downstream matmuls.
        def permute(dstT, srcT, fwd, da_order=(0, 1, 2, 3)):
            sv = srcT[:].rearrange("p k (a b i j) -> p k a b i j", a=G, b=G, i=WS, j=WS)
            dv = dstT[:].rearrange("p k (a b i j) -> p k a b i j", a=G, b=G, i=WS, j=WS)
            groups = {a: [] for a in range(G)}
            for qa in range(2):
                di = slice(0, 4) if qa == 0 else slice(4, 8)
                si = slice(4, 8) if qa == 0 else slice(0, 4)
                for qb in range(2):
                    dj = slice(0, 4) if qb == 0 else slice(4, 8)
                    sj = slice(4, 8) if qb == 0 else slice(0, 4)
                    if qb == 0:
                        bpairs = [(slice(0, G), slice(0, G))]
                    else:
                        bpairs = [(slice(0, G - 1), slice(1, G)),
                                  (slice(G - 1, G), slice(0, 1))]
                    for a in range(G):
                        sa = (a + qa) % G
                        for db, sb_ in bpairs:
                            if fwd:
                                groups[a].append(((a, db, di, dj), (sa, sb_, si, sj)))
                            else:
                                groups[sa].append(((sa, sb_, si, sj), (a, db, di, dj)))
            n = 0
            for da in da_order:
                for (dd, ss) in groups[da]:
                    for ct in range(KC):
                        eng = (nc.gpsimd, nc.vector)[n % 2]
                        n += 1
                        eng.tensor_copy(dv[:, ct, dd[0], dd[1], dd[2], dd[3]],
                                        sv[:, ct, ss[0], ss[1], ss[2], ss[3]])

        qkT = qkp.tile([P, 8, L], BF16)
        v_aug = vp.tile([P, NT, VW], BF16)
        att = attp.tile([P, NT, C], BF16)

        # ---------- layers ----------
        for i in range(D):
            shift = (i % 2) == 1
            if _STAGE < 1:
                continue
            # LN1 + transpose (+ shift permute)
            h = hp.tile([P, NT, C], BF16, name="h")
            layernorm(h, x, "l1")
            prefetch_act(AF.Exp)
            if _STAGE < 2:
                continue
            with tc.tile_pool(name="tp1", bufs=2, space="PSUM") as tpool:
                hT_A = hTp.tile([P, KC, L], BF16, name="hT")
                transpose_to(hT_A, h, tpool)
            if shift:
                hT = hTp.tile([P, KC, L], BF16, name="hT")
                permute(hT, hT_A, True)
            else:
                hT = hT_A
            if _STAGE < 3:
                continue

            # ---- qk + v ----
            wqk_sb = wqkp.tile([P, KC, 2 * C], BF16, name="wqk")
            nc.sync.dma_start(wqk_sb[:], dr['wqk'][i].rearrange("(k p) m -> p k m", p=P))
            wvp_sb = wvpp.tile([P, KC, VW], BF16, name="wvp")
            nc.sync.dma_start(wvp_sb[:], dr['wvp'][i].rearrange("(k p) m -> p k m", p=P))
            vb_sb = bp.tile([P, VW], F32, name="vb", tag="vb")
            nc.sync.dma_start(vb_sb[:], dr['vb'][i])
            bqk_sb = None
            if not flags['bqk0']:
                bqk_sb = bp.tile([P, 8], F32, name="bqk", tag="bqk")
                nc.sync.dma_start(bqk_sb[:], dr['bqk'][i])
            with tc.tile_pool(name="qkps", bufs=3, space="PSUM") as qkps, \
                 tc.tile_pool(name="vps", bufs=2, space="PSUM") as vps:
                for tc2 in range(2):
                    for mo in range(8):
                        ps = qkps.tile([P, C], F32, name="qkmm")
                        for kk in range(KC):
                            nc.tensor.matmul(ps[:], wqk_sb[:, kk, mo * P:(mo + 1) * P],
                                             hT[:, kk, tc2 * 512:(tc2 + 1) * 512],
                                             start=(kk == 0), stop=(kk == KC - 1))
                        if flags['bqk0']:
                            if mo % 2 == 0:
                                nc.scalar.copy(qkT[:, mo, tc2 * 512:(tc2 + 1) * 512], ps[:])
                            else:
                                nc.vector.tensor_copy(qkT[:, mo, tc2 * 512:(tc2 + 1) * 512],
                                                      ps[:])
                        else:
                            nc.scalar.activation(qkT[:, mo, tc2 * 512:(tc2 + 1) * 512],
                                                 ps[:], AF.Identity,
                                                 bias=bqk_sb[:, mo:mo + 1])
                for tt in range(NT):
                    ps = vps.tile([P, VW], F32, name="vmm")
                    for kk in range(KC):
                        nc.tensor.matmul(ps[:, 0:512], hT[:, kk, tt * P:(tt + 1) * P],
                                         wvp_sb[:, kk, 0:512], start=(kk == 0),
                                         stop=(kk == KC - 1), skip_group_check=True)
                        nc.tensor.matmul(ps[:, 512:VW], hT[:, kk, tt * P:(tt + 1) * P],
                                         wvp_sb[:, kk, 512:VW], start=(kk == 0),
                                         stop=(kk == KC - 1), skip_group_check=True)
                    nc.vector.tensor_add(v_aug[:, tt], ps[:], vb_sb[:])

            if _STAGE < 4:
                continue
            # ---- attention ----
            # EE tiles for this layer
            pts_needed = sorted(set(_WP2_PT)) if shift else [0]
            ee_sb = {}
            for pt in pts_needed:
                t = eep.tile([P, 4, 256], BF16, name=f"ee{pt}", tag=f"ee{pt}")
                nc.sync.dma_start(t[:], dr['ee'][i, pt].rearrange("r p c -> p r c"))
                for rg in range(4):
                    ee_sb[(pt, rg)] = t[:, rg]
            with tc.tile_pool(name="sps", bufs=1, space="PSUM") as sps, \
                 tc.tile_pool(name="avps", bufs=2, space="PSUM") as avps:
                for wp2 in range(NT):
                    pt = _WP2_PT[wp2] if shift else 0
                    # S: one full PSUM bank per rg (row-group); MMs from different
                    # row-groups must not share a bank. rg-interleaved emission so
                    # LDWs pull ahead across row groups.
                    sgs = [sps.tile([P, 8, NW], F32, name=f"s{rg}", tag=f"s{rg}")
                           for rg in range(4)]
                    for j in range(8):
                        hi = j % 4
                        w01 = j // 4
                        wc = slice((wp2 * 2 + w01) * NW, (wp2 * 2 + w01 + 1) * NW)
                        for rg in range(4):
                            nc.tensor.matmul(
                                sgs[rg][w01 * NW:(w01 + 1) * NW, hi, :],
                                qkT[rg * HD:(rg + 1) * HD, 4 + hi, wc],
                                qkT[rg * HD:(rg + 1) * HD, hi, wc],
                                start=True, stop=True, skip_group_check=True,
                                tile_position=(rg * HD, w01 * NW))
                    if _ATT < 1:
                        continue
                    ptg = []
                    for rg in range(4):
                        er = ptp.tile([P, 4, NW], BF16, name=f"er{rg}", tag=f"er{rg}")
                        nc.scalar.activation(er[:].rearrange("p a b -> p (a b)"),
                                             sgs[rg][:, 0:4, :].rearrange("p a b -> p (a b)"),
                                             AF.Exp)
                        pt_t = ptp.tile([P, 4, NW], BF16, name=f"pt{rg}", tag=f"pt{rg}")
                        nc.gpsimd.tensor_mul(pt_t[:].rearrange("p a b -> p (a b)"),
                                             er[:].rearrange("p a b -> p (a b)"),
                                             ee_sb[(pt, rg)])
                        ptg.append(pt_t)
                    if _ATT < 2:
                        continue
                    avs = [avps.tile([P, 8, NW], F32, name=f"av{hs}", tag=f"av{hs}")
                           for hs in range(2)]
                    for s in range(8):
                        hi0 = (s // 4) % 2
                        rg = s % 4
                        for hs in range(2):
                            hi = 2 * hs + hi0
                            h_ = hi * 4 + rg
                            for w01 in range(2):
                                rows = slice(w01 * NW, (w01 + 1) * NW)
                                nc.tensor.matmul(
                                    avs[hs][rows, s, 0:VBLK], ptg[rg][rows, hi, :],
                                    v_aug[rows, wp2, h_ * VBLK:(h_ + 1) * VBLK],
                                    start=True, stop=True, skip_group_check=True,
                                    tile_position=(w01 * NW, w01 * NW))
                    if _ATT < 3:
                        continue
                    for hs in range(2):
                        rec = recp.tile([P, 8], F32, name="rec", tag=f"rec{hs}")
                        nc.vector.reciprocal(rec[:], avs[hs][:, :, HD])
                        rb = rec[:].rearrange("p (a b) -> p a b", b=1) \
                            .to_broadcast((P, 8, HD))
                        dst = att[:, wp2, hs * 256:(hs + 1) * 256] \
                            .rearrange("p (a b) -> p a b", b=HD)
                        nc.vector.tensor_mul(dst, avs[hs][:, :, 0:HD], rb)

            if _STAGE < 5:
                continue
            prefetch_act(AF.Sqrt)
            # ---- attn transpose back (+ inverse shift permute) + proj ----
            with tc.tile_pool(name="tp2", bufs=2, space="PSUM") as tpool:
                aT_B = hTp.tile([P, KC, L], BF16, name="hT")
                transpose_to(aT_B, att, tpool)
            if shift:
                aT = hTp.tile([P, KC, L], BF16, name="hT")
                permute(aT, aT_B, False, da_order=(1, 2, 3, 0))
            else:
                aT = aT_B
            wp_sb = wpp.tile([P, KC, C], BF16, name="wp")
            nc.sync.dma_start(wp_sb[:], dr['wp'][i].rearrange("(k p) m -> p k m", p=P))
            if not flags['pb0']:
                pb_sb = bp.tile([P, C], F32, name="pb", tag="pb")
                nc.sync.dma_start(pb_sb[:], dr['pb'][i])
            with tc.tile_pool(name="pjps", bufs=3, space="PSUM") as pjps:
                tt_order = (2, 3, 4, 5, 6, 7, 0, 1) if shift else range(NT)
                for tt in tt_order:
                    ps = pjps.tile([P, C], F32, name="pjmm")
                    for kk in range(KC):
                        nc.tensor.matmul(ps[:], aT[:, kk, tt * P:(tt + 1) * P],
                                         wp_sb[:, kk, :], start=(kk == 0), stop=(kk == KC - 1))
                    nc.vector.tensor_add(x[:, tt], ps[:], x[:, tt])
                    if not flags['pb0']:
                        nc.gpsimd.tensor_add(x[:, tt], x[:, tt], pb_sb[:])

            if _STAGE < 6:
                continue
            # ---- LN2 + transpose + MLP ----
            h2 = hp.tile([P, NT, C], BF16, name="h")
            layernorm(h2, x, "l2")
            prefetch_act(AF.Gelu)
            with tc.tile_pool(name="tp3", bufs=2, space="PSUM") as tpool:
                h2T = hTp.tile([P, KC, L], BF16, name="hT")
                transpose_to(h2T, h2, tpool)
            if _STAGE < 7:
                continue
            f1b_sb = bp.tile([P, FH // P], F32, name="f1b", tag="f1b")
            nc.sync.dma_start(f1b_sb[:], dr['f1b'][i])
            if not flags['f2b0']:
                f2b_sb = bp.tile([P, C], F32, name="f2b", tag="f2b")
                nc.sync.dma_start(f2b_sb[:], dr['f2b'][i])
            with tc.tile_pool(name="f1ps", bufs=3, space="PSUM") as f1ps, \
                 tc.tile_pool(name="fc2ps", bufs=1, space="PSUM") as fc2ps:
                for tc2 in range(2):
                    pso = [fc2ps.tile([P, C], F32, name=f"fc2_{j}", tag=f"fc2_{j}")
                           for j in range(4)]
                    for q in range(4):
                        f1c = f1p.tile([P, KC, 4 * P], BF16, name="f1c")
                        nc.sync.dma_start(f1c[:], dr['f1'][i][:, q * 512:(q + 1) * 512]
                                          .rearrange("(k p) m -> p k m", p=P))
                        f2c = f2p.tile([P, 4, C], BF16, name="f2c")
                        nc.sync.dma_start(f2c[:], dr['f2'][i][q * 512:(q + 1) * 512, :]
                                          .rearrange("(o p) c -> p o c", p=P))
                        for oq in range(4):
                            ho = q * 4 + oq
                            ps1 = f1ps.tile([P, C], F32, name="f1mm")
                            for kk in range(KC):
                                nc.tensor.matmul(ps1[:], f1c[:, kk, oq * P:(oq + 1) * P],
                                                 h2T[:, kk, tc2 * 512:(tc2 + 1) * 512],
                                                 start=(kk == 0), stop=(kk == KC - 1))
                            g = gp.tile([P, C], BF16, name="g")
                            nc.scalar.activation(g[:], ps1[:], AF.Gelu, bias=f1b_sb[:, ho:ho + 1])
                            for j in range(4):
                                nc.tensor.matmul(pso[j][:], g[:, j * P:(j + 1) * P],
                                                 f2c[:, oq, :],
                                                 start=(ho == 0), stop=(ho == FH // P - 1))
                    if tc2 == 1:
                        prefetch_act(AF.Sqrt)
                    for j in range(4):
                        tt = tc2 * 4 + j
                        nc.vector.tensor_add(x[:, tt], pso[j][:], x[:, tt])
                        if not flags['f2b0']:
                            nc.gpsimd.tensor_add(x[:, tt], x[:, tt], f2b_sb[:])

        # ---------- final LN + gelu + pred ----------
        hf = hp.tile([P, NT, C], BF16, name="h")
        layernorm(hf, x, "lf")
        with tc.tile_pool(name="tpf", bufs=2, space="PSUM") as tpool:
            hfT = hTp.tile([P, KC, L], BF16, name="hT")
            transpose_to(hfT, hf, tpool)
        nfw_sb = bp.tile([P, KC], F32, name="nfw", tag="nfw")
        nc.sync.dma_start(nfw_sb[:], dr['nfw'])
        nfb_sb = bp.tile([P, KC], F32, name="nfb", tag="nfb")
        nc.sync.dma_start(nfb_sb[:], dr['nfb'])
        gT = hTp.tile([P, KC, L], BF16, name="gT", bufs=1)
        for ct in range(KC):
            nc.scalar.activation(gT[:, ct], hfT[:, ct], AF.Gelu,
                                 bias=nfb_sb[:, ct:ct + 1], scale=nfw_sb[:, ct:ct + 1])
        with tc.tile_pool(name="mmpsf", bufs=4, space="PSUM") as mmps:
            for n4 in range(N_E // (4 * P)):
                pwc = pwp.tile([P, KC, 4 * P], BF16, name="pwc")
                nc.sync.dma_start(pwc[:], dr['pw'][:, n4 * 512:(n4 + 1) * 512]
                                  .rearrange("(k p) m -> p k m", p=P))
                for nq in range(4):
                    no = n4 * 4 + nq
                    osb = outp.tile([P, L], BF16, name="osb")
                    for tc2 in range(2):
                        ps = mmps.tile([P, 512], F32, name="pmm")
                        for kk in range(KC):
                            nc.tensor.matmul(ps[:], pwc[:, kk, nq * P:(nq + 1) * P],
                                             gT[:, kk, tc2 * 512:(tc2 + 1) * 512],
                                             start=(kk == 0), stop=(kk == KC - 1))
                        if no % 2 == 0:
                            nc.scalar.copy(osb[:, tc2 * 512:(tc2 + 1) * 512], ps[:])
                        else:
                            nc.vector.tensor_copy(osb[:, tc2 * 512:(tc2 + 1) * 512], ps[:])
                    nc.sync.dma_start(outT[no * P:(no + 1) * P, :], osb[:])

    nc.compile()
    _BUILD_CACHE[key] = nc
    return nc


LAST_RESULTS = None


def kernel(**inputs):
    global LAST_RESULTS
    from concourse import bass_utils
    sh, xts, pred_b, flags = _prepare(inputs)
    nc = _build(flags)
    in_maps = []
    for c in range(_NCORES):
        m = dict(sh)
        m['xT'] = xts[c % B]
        in_maps.append(m)
    trace = os.environ.get("BT_TRACE", "0") == "1"
    if trace:
        try:
            import antenv.axon_hooks  # noqa: F401
        except ImportError:
            trace = False
    res = bass_utils.run_bass_kernel_spmd(nc, in_maps, core_ids=list(range(_NCORES)),
                                          trace=trace)
    LAST_RESULTS = res
    outs = []
    for c in range(B):
        oT = np.asarray(res.results[c % _NCORES]['outT'], dtype=np.float32)  # [N_E, L]
        o = oT.T[_WM_INV] + pred_b[None, :]        # [L, N_E] raster order
        outs.append(o)
    return np.stack(outs).astype(np.float32)
